# revision 35
# baseline (speedup 1.0000x reference)
"""Trainium2 Bass kernel for nn_AttnBlock_16887811407979 (sparse attention).

Strategy: 8-way sequence-parallel SPMD (each core handles a 256-query
slice, all heads), no collectives. The sparse gather is densified: the
host converts (attendable_indices, valid_indices_mask) into a dense
count matrix C[n, q], so softmax-over-slots == count-weighted dense
softmax: W[n,q] = C[n,q]*exp(S^T[n,q]); O[q] = (W^T V)/sum_n W[n,q].

v3:
  * x ships as bf16 (2MB), GN stats on bf16, affine produces f32r h for
    the K/Q/V convs (score path keeps f32r precision); V/attention
    weights stay bf16.
  * DMA priority ordering on the two HWDGE rings (sync+scalar): x
    column-halves first, then xq, wk, wq, wv, cnt, wpb - so x gets full
    aggregate bandwidth and GroupNorm starts ~8us earlier.
  * per-chunk GN stat matmuls issued as col0/col1 pairs (shared-bank
    accumulation) so they retire as each x chunk lands.
  * attention: one [128,1024] exp per chunk (ACT was the pass
    bottleneck at two exps), O^T accumulators 2 banks/pass via
    shared-bank start/stop, scores double-buffered in 4 banks.
  * softmax reciprocal = exp(-ln(s)) on ACT (DVE reciprocal is 6.5us),
    broadcast via GpSimd partition_broadcast (idle engine), normalize
    folded into a DVE mult; pass-0 chain overlaps pass 1.
  * tail: first half of projection runs while pass-1 normalization
    chain completes; epilogue pipelined per channel chunk.
"""
import sys
import types
import contextlib

sys.path.insert(0, '/opt/trn_rl_repo')
sys.path.insert(0, '/root/.axon_site')

import numpy as np
import ml_dtypes

import concourse.bass as bass
import concourse.tile as tile
from concourse import mybir
from concourse.vector_clock import ScopedClock
from concourse.bass_utils import run_bass_kernel_spmd

f32 = mybir.dt.float32
f32r = mybir.dt.float32r
bf16 = mybir.dt.bfloat16
AF = mybir.ActivationFunctionType
AX = mybir.AxisListType
ALU = mybir.AluOpType

N_CORES = 8
C = 512
N = 2048
HEADS = 8
D = 64
K_IDX = 128
GROUPS = 32
GSIZE = C // GROUPS          # 16 channels per group
NQ = N // N_CORES            # 256 queries per core
NCHUNK = N // 128            # 16 key chunks
CCHUNK = C // 128            # 4 channel chunks
EPS = 1e-6
USE_GP_BCAST = False         # walrus here lacks custom GpSimd ISA ops

# Attention runs in two passes of 4 heads. global block b = 4*pass + lb:
BLK = [4 * (h // 4) + (h % 4) // 2 + 2 * (h % 2) for h in range(HEADS)]
HB = [0] * 8
for _h in range(HEADS):
    HB[BLK[_h]] = _h                                     # b -> h

# ---------------------------------------------------------------------------
# walrus workaround: this container's walrus accepts at most ONE embedded
# sync-wait per engine instruction. Split Tile's multi-wait instructions
# into chains of single-wait NoOps, and do the same for the kernel-tail
# drain that Tile emits at TileContext exit.
# ---------------------------------------------------------------------------
_wsplit = [0]


def _drain_and_barrier_split(self, tick_clock, wait_clock):
    nc = self.nc
    carrier = nc.sync.nop(nofuse=True)
    wait_clock.add_sem_waits(
        carrier.ins, ScopedClock({None: tick_clock.global_clock}))
    si = carrier.ins.sync_info
    waits = list(si.on_wait or []) if si is not None else []
    if len(waits) > 1:
        carrier.ins.sync_info = mybir.SyncInfo(
            on_wait=waits[:1], on_update=list(si.on_update or []))
        for w in waits[1:]:
            extra = nc.sync.nop(nofuse=True)
            extra.ins.sync_info = mybir.SyncInfo(on_wait=[w], on_update=[])
    nc.sync.drain()
    nc.all_engine_barrier(sem_only=True)
    assert self.sems is not None
    popped = nc._tile_sem_poison_stack.pop()
    assert popped is self._sem_poison
    nc.clear_and_free_semaphores(list(self.sems.allocated().values()))
    nc.all_engine_barrier(sem_only=True)


def _split_sync_waits(nc, max_waits=1):
    for f in nc.m.functions:
        for bb in f.blocks:
            insts = bb.instructions
            out = []
            changed = False
            for inst in insts:
                si = inst.sync_info
                waits = list(si.on_wait or []) if si is not None else []
                if len(waits) > max_waits:
                    changed = True
                    for i in range(len(waits) - max_waits):
                        _wsplit[0] += 1
                        nop = mybir.InstNoOp(
                            name=f"I-wsplit-{_wsplit[0]}", ins=[], outs=[])
                        nop.engine = inst.engine
                        nop.sync_info = mybir.SyncInfo(
                            on_wait=[waits[i]], on_update=[])
                        out.append(nop)
                    inst.sync_info = mybir.SyncInfo(
                        on_wait=waits[len(waits) - max_waits:],
                        on_update=list(si.on_update or []))
                out.append(inst)
            if changed:
                if isinstance(insts, list):
                    insts[:] = out
                else:
                    bb.instructions = out


tile.TileContext._drain_and_barrier = _drain_and_barrier_split


# ---------------------------------------------------------------------------
# kernel builder
# ---------------------------------------------------------------------------

def _build(split_waits=True):
    nc = bass.Bass("TRN2", target_bir_lowering=False, debug=False)

    def din(name, shape, dt=f32):
        return nc.dram_tensor(name, shape, dt, kind="ExternalInput").ap()

    x_d = din("x", [C, N], bf16)
    xq_d = din("xq", [C, NQ])
    cnt_d = din("cnt", [N, NQ], bf16)
    wkT_d = din("wkT", [C, C])
    wqT_d = din("wqT", [C, C])
    wvT_d = din("wvT", [C, C])
    wpTb_d = din("wpTb", [C, C], bf16)
    smalls_d = din("smalls", [128, 20])
    bvrow_d = din("bvrow", [1, C], bf16)
    gind_d = din("gind", [128, 32 * CCHUNK])
    gindT_d = din("gindT", [GROUPS, C])
    out_d = nc.dram_tensor("out", [C, NQ], f32, kind="ExternalOutput").ap()

    with tile.TileContext(nc) as tc, contextlib.ExitStack() as ctx:
        P = ctx.enter_context(tc.tile_pool(name="persist", bufs=1))
        A = ctx.enter_context(tc.tile_pool(name="phase_a", bufs=1))

        # ---- DMAs in priority order on both HWDGE rings ----
        # x column-halves first (GN stats are the critical path), then
        # xq (hq affine gates qconv), then wk (kconv is first big conv),
        # wq, wv, then cnt/wpb (needed much later). Tiny tensors ride
        # along early.
        xt = [A.tile([128, N], bf16, tag=f"xt{k}", name=f"xt{k}")
              for k in range(CCHUNK)]
        for k in range(CCHUNK):
            nc.sync.dma_start(xt[k][:], x_d[128 * k:128 * (k + 1), :])
        smallst = P.tile([128, 20], f32, tag="smalls", name="smalls")
        nc.sync.dma_start(smallst[:], smalls_d)
        gindt = P.tile([128, 32 * CCHUNK], f32, tag="gind", name="gind")
        nc.sync.dma_start(gindt[:], gind_d)
        gindTt = P.tile([GROUPS, C], f32, tag="gindT", name="gindT")
        nc.sync.dma_start(gindTt[:], gindT_d)
        bvrow_t = P.tile([1, C], bf16, tag="bvrow", name="bvrow")
        nc.sync.dma_start(bvrow_t[:], bvrow_d)
        xqt = P.tile([128, NQ * CCHUNK], f32, tag="xqt", name="xqt")
        nc.sync.dma_start(
            xqt[:].rearrange("p (k q) -> p k q", k=CCHUNK),
            xq_d.rearrange("(k p) q -> p k q", p=128))

        wkt = P.tile([128, C * CCHUNK], f32r, tag="wkt", name="wkt")
        nc.gpsimd.dma_start(
            wkt[:].rearrange("p (k c) -> p k c", k=CCHUNK),
            wkT_d.rearrange("(k p) c -> p k c", p=128))
        wqt = P.tile([128, C * CCHUNK], f32r, tag="wqt", name="wqt")
        nc.gpsimd.dma_start(
            wqt[:].rearrange("p (k c) -> p k c", k=CCHUNK),
            wqT_d.rearrange("(k p) c -> p k c", p=128))
        wvt = P.tile([128, C * CCHUNK], f32r, tag="wvt", name="wvt")
        nc.gpsimd.dma_start(
            wvt[:].rearrange("p (k c) -> p k c", k=CCHUNK),
            wvT_d.rearrange("(k p) c -> p k c", p=128))
        cntt = P.tile([128, 256 * NCHUNK], bf16, tag="cntt", name="cntt")
        nc.sync.dma_start(
            cntt[:].rearrange("p (m q) -> p m q", m=NCHUNK),
            cnt_d.rearrange("(m p) q -> p m q", p=128))
        wpb = P.tile([64, C * HEADS], bf16, tag="wpb", name="wpb")
        nc.sync.dma_start(
            wpb[:].rearrange("p (b c) -> p b c", b=HEADS),
            wpTb_d.rearrange("(b p) c -> p b c", p=64))

        h32 = [P.tile([128, N], f32r, tag=f"h32{k}", name=f"h32{k}")
               for k in range(CCHUNK)]
        hq32 = P.tile([128, NQ * CCHUNK], f32r, tag="hq32", name="hq32")

        def sm(k, f):
            return smallst[:, 5 * k + f:5 * k + f + 1]

        # ---- GroupNorm stats -> per-channel scale A / bias B ----
        # per-chunk partial sums over the two column halves (so work
        # starts as each half lands), combined in f32.
        s1h = [[P.tile([128, 1], f32, tag=f"s1h{k}_{j}", name=f"s1h{k}_{j}")
                for j in range(2)] for k in range(CCHUNK)]
        s2h = [[P.tile([128, 1], f32, tag=f"s2h{k}_{j}", name=f"s2h{k}_{j}")
                for j in range(2)] for k in range(CCHUNK)]
        s12 = [P.tile([128, 2], f32, tag=f"s12{k}", name=f"s12{k}")
               for k in range(CCHUNK)]
        At = [P.tile([128, 1], f32, tag=f"A{k}", name=f"A{k}")
              for k in range(CCHUNK)]
        Bt = [P.tile([128, 1], f32, tag=f"B{k}", name=f"B{k}")
              for k in range(CCHUNK)]
        with tc.tile_pool(name="gnps", bufs=1, space="PSUM") as gnps:
            for k in range(CCHUNK):
                for j in range(2):
                    half = xt[k][:, 1024 * j:1024 * (j + 1)]
                    nc.vector.tensor_reduce(s1h[k][j][:], half,
                                            axis=AX.X, op=ALU.add)
                    sq = A.tile([128, 1024], bf16, tag="sq", name="sq")
                    nc.scalar.activation(sq[:], half, AF.Square,
                                         accum_out=s2h[k][j][:])
                nc.vector.tensor_add(s12[k][:, 0:1], s1h[k][0][:],
                                     s1h[k][1][:])
                nc.vector.tensor_add(s12[k][:, 1:2], s2h[k][0][:],
                                     s2h[k][1][:])
            # gs accumulation: per chunk, col0 (sum) and col1 (sumsq)
            # share a bank; single start=True on the very first matmul.
            gs = gnps.tile([GROUPS, 2], f32, tag="gs", name="gs")
            for k in range(CCHUNK):
                for c2 in range(2):
                    nc.tensor.matmul(gs[:, c2:c2 + 1],
                                     gindt[:, 32 * k:32 * (k + 1)],
                                     s12[k][:, c2:c2 + 1],
                                     start=(k == 0 and c2 == 0),
                                     stop=(k == CCHUNK - 1 and c2 == 1),
                                     skip_group_check=True)
            mstat = P.tile([GROUPS, 2], f32, tag="mstat", name="mstat")
            inv_n = 1.0 / (GSIZE * N)
            nc.vector.tensor_scalar_mul(mstat[:, 0:1], gs[:, 0:1], inv_n)
            msq = P.tile([GROUPS, 1], f32, tag="msq", name="msq")
            nc.vector.tensor_scalar_mul(msq[:], gs[:, 1:2], inv_n)
            m2 = P.tile([GROUPS, 1], f32, tag="m2", name="m2")
            nc.vector.tensor_mul(m2[:], mstat[:, 0:1], mstat[:, 0:1])
            var = P.tile([GROUPS, 1], f32, tag="var", name="var")
            nc.vector.tensor_sub(var[:], msq[:], m2[:])
            nc.vector.tensor_scalar_add(var[:], var[:], float(EPS))
            std = P.tile([GROUPS, 1], f32, tag="std", name="std")
            nc.scalar.activation(std[:], var[:], AF.Sqrt)
            nc.vector.reciprocal(mstat[:, 1:2], std[:])
            mr = [P.tile([128, 2], f32, tag=f"mr{k}", name=f"mr{k}")
                  for k in range(CCHUNK)]
            for k in range(CCHUNK):
                mrp = gnps.tile([128, 2], f32, tag="mrp", name="mrp", bufs=2)
                nc.tensor.matmul(mrp[:], gindTt[:, 128 * k:128 * (k + 1)],
                                 mstat[:], start=True, stop=True)
                nc.vector.tensor_copy(mr[k][:], mrp[:])
            for k in range(CCHUNK):
                nc.vector.tensor_mul(At[k][:], sm(k, 3), mr[k][:, 1:2])
                tmp = P.tile([128, 1], f32, tag="tmpB", name="tmpB")
                nc.vector.tensor_mul(tmp[:], mr[k][:, 0:1], At[k][:])
                nc.vector.tensor_sub(Bt[k][:], sm(k, 4), tmp[:])
            # hq affine first (gates qconv), then h chunks (ACT/DVE split)
            for k in range(CCHUNK):
                nc.scalar.activation(hq32[:, NQ * k:NQ * (k + 1)],
                                     xqt[:, NQ * k:NQ * (k + 1)],
                                     AF.Identity,
                                     bias=Bt[k][:, 0:1], scale=At[k][:, 0:1])
            for k in range(CCHUNK):
                if k % 2 == 0:
                    nc.scalar.activation(h32[k][:], xt[k][:], AF.Identity,
                                         bias=Bt[k][:, 0:1],
                                         scale=At[k][:, 0:1])
                else:
                    with nc.allow_low_precision(reason="f32r affine"):
                        nc.vector.tensor_scalar(
                            h32[k][:], xt[k][:],
                            At[k][:, 0:1], Bt[k][:, 0:1],
                            op0=ALU.mult, op1=ALU.add)

        # residual + proj bias, precombined for the tail epilogue
        xqb = P.tile([128, NQ * CCHUNK], f32, tag="xqb", name="xqb")
        for k in range(CCHUNK):
            nc.scalar.activation(xqb[:, NQ * k:NQ * (k + 1)],
                                 xqt[:, NQ * k:NQ * (k + 1)],
                                 AF.Identity, bias=sm(k, 2))

        kt = [P.tile([128, N], f32r, tag=f"kt{k}", name=f"kt{k}")
              for k in range(CCHUNK)]
        qt = [P.tile([128, NQ], f32r, tag=f"qt{k}", name=f"qt{k}")
              for k in range(CCHUNK)]
        vt = [P.tile([128, 65 * HEADS], bf16, tag=f"vt{m}", name=f"vt{m}")
              for m in range(NCHUNK)]
        on = P.tile([64, 256 * HEADS], bf16, tag="on", name="on")
        tln = P.tile([1, 256 * HEADS], f32, tag="tln", name="tln")
        rrb = P.tile([1, 256 * HEADS], bf16, tag="rrb", name="rrb")
        rbb = P.tile([64, 256 * HEADS], bf16, tag="rbb", name="rbb")
        ones1 = P.tile([1, 128], bf16, tag="ones1", name="ones1")
        nc.vector.memset(ones1[:], 1.0)

        LN2_32 = float(32 * np.log(2.0))
        bias_ln = P.tile([1, 1], f32, tag="bias_ln", name="bias_ln")
        nc.vector.memset(bias_ln[:], -LN2_32)

        def recip_chain(p, ot):
            # 1/rowsums: ln on ACT, exp(-t) on ACT (off the PE/DVE
            # critical path). Rowsums reach ~e^52 which overflows the
            # Ln table's range (~2^64): feed s*2^-32 and compensate in
            # the exp bias: 1/s = exp(-(ln(s*2^-32)) - 32*ln2).
            sl = slice(1024 * p, 1024 * (p + 1))
            nc.scalar.activation(tln[0:1, sl], ot[64:65, :], AF.Ln,
                                 scale=float(2.0 ** -32))
            nc.scalar.activation(rrb[0:1, sl], tln[0:1, sl], AF.Exp,
                                 scale=-1.0, bias=bias_ln[:, 0:1])
            if USE_GP_BCAST:
                nc.gpsimd.partition_broadcast(rbb[:, sl], rrb[0:1, sl])

        def norm_mult(p, ot):
            sl = slice(1024 * p, 1024 * (p + 1))
            with nc.allow_low_precision(reason="bf16 on"):
                nc.vector.tensor_mul(on[:, sl], ot[0:64, :], rbb[:, sl])

        # ---- dense conv phase (keeps PE warm), then attention ----
        with tc.tile_pool(name="asb", bufs=2) as asb:
            cps_cm = tc.tile_pool(name="cps", bufs=4, space="PSUM")
            cps = cps_cm.__enter__()

            def kconv_colgroup(j):
                cols = slice(512 * j, 512 * (j + 1))
                for m in range(CCHUNK):
                    pk = cps.tile([128, 512], f32, tag="cp", name="cpk")
                    for ci in range(CCHUNK):
                        nc.tensor.matmul(
                            pk[:],
                            wkt[:, C * ci + 128 * m:C * ci + 128 * (m + 1)],
                            h32[ci][:, cols],
                            start=(ci == 0), stop=(ci == CCHUNK - 1))
                    if m % 2 == 0:
                        with nc.allow_low_precision(reason="f32r k"):
                            nc.scalar.activation(kt[m][:, cols], pk[:],
                                                 AF.Identity, bias=sm(m, 0))
                    else:
                        with nc.allow_low_precision(reason="f32r k"):
                            nc.vector.tensor_scalar_add(kt[m][:, cols],
                                                        pk[:], sm(m, 0))

            def qconv():
                for m in range(CCHUNK):
                    pq = cps.tile([128, 512], f32, tag="cp",
                                  name="cpq")[:, 0:NQ]
                    for ci in range(CCHUNK):
                        nc.tensor.matmul(
                            pq[:],
                            wqt[:, C * ci + 128 * m:C * ci + 128 * (m + 1)],
                            hq32[:, NQ * ci:NQ * (ci + 1)],
                            start=(ci == 0), stop=(ci == CCHUNK - 1))
                    with nc.allow_low_precision(reason="f32r q"):
                        nc.scalar.activation(qt[m][:], pq[:], AF.Identity,
                                             bias=sm(m, 1))

            def vconv(m):
                pv = cps.tile([128, C], f32, tag="cp", name="cpv")
                for ci in range(CCHUNK):
                    nc.tensor.matmul(pv[:],
                                     h32[ci][:, 128 * m:128 * (m + 1)],
                                     wvt[:, C * ci:C * (ci + 1)],
                                     start=(ci == 0),
                                     stop=(ci == CCHUNK - 1))
                dst = vt[m][:].rearrange("p (h e) -> p h e",
                                         h=HEADS)[:, :, 0:64]
                nc.vector.scalar_tensor_tensor(
                    dst, pv[:].rearrange("p (h d) -> p h d", h=HEADS), 1.0,
                    bvb[:].rearrange("p (h d) -> p h d", h=HEADS),
                    op0=ALU.mult, op1=ALU.add)
                ones_cols = vt[m][:].rearrange(
                    "p (h e) -> p h e", h=HEADS)[:, :, 64:65]
                nc.gpsimd.memset(ones_cols, 1.0)

            def attn_scores(p, m):
                st = sps.tile([128, 1024], f32, tag="st", name=f"st{p}_{m}")
                for h in range(4 * p, 4 * p + 4):
                    par = h % 2
                    cm = h // 2
                    lb = BLK[h] - 4 * p
                    nc.tensor.matmul(
                        st[:, 256 * lb:256 * (lb + 1)],
                        kt[cm][64 * par:64 * (par + 1),
                               128 * m:128 * (m + 1)],
                        qt[cm][64 * par:64 * (par + 1), :],
                        start=True, stop=True)
                et = asb.tile([128, 1024], bf16, tag="et", name=f"et{p}_{m}")
                nc.scalar.activation(et[:], st[:], AF.Exp)
                wt = asb.tile([128, 1024], bf16, tag="wt", name=f"wt{p}_{m}")
                nc.vector.tensor_mul(
                    wt[:].rearrange("p (b q) -> p b q", b=4),
                    et[:].rearrange("p (b q) -> p b q", b=4),
                    cntt[:, 256 * m:256 * (m + 1)].unsqueeze(1)
                        .broadcast_to([128, 4, NQ]))
                return wt

            def attn_ov(p, m, ot, wt):
                for h in range(4 * p, 4 * p + 4):
                    lb = BLK[h] - 4 * p
                    nc.tensor.matmul(
                        ot[0:65, 256 * lb:256 * (lb + 1)],
                        vt[m][:, 65 * h:65 * h + 65],
                        wt[:, 256 * lb:256 * (lb + 1)],
                        start=(m == 0 and lb % 2 == 0),
                        stop=(m == NCHUNK - 1 and lb % 2 == 1),
                        skip_group_check=True)

            # dense conv burst: K first (wk lands first), Q, then V groups
            kconv_colgroup(0)
            qconv()
            # bv broadcast [128, C] (bf16 matmul; needed by first vconv)
            bvb = P.tile([128, C], f32, tag="bvb", name="bvb")
            pbv2 = cps.tile([128, C], f32, tag="cp", name="cpbv2")
            nc.tensor.matmul(pbv2[:], ones1[:], bvrow_t[:], start=True,
                             stop=True)
            nc.vector.tensor_copy(bvb[:], pbv2[:])
            for j in range(1, 4):
                kconv_colgroup(j)
                for m in range(4 * (j - 1), 4 * j):
                    vconv(m)
            for m in range(12, 16):
                vconv(m)
            cps_cm.__exit__(None, None, None)

            # attention pools: ot0/ot1 2 banks each, scores 2x2 banks.
            # stack: ops0, ops1 stay open through the tail normalize;
            # sps closes after pass 1 to free 4 banks for rps+pps.
            ops0_cm = tc.tile_pool(name="ops0", bufs=1, space="PSUM")
            ops0 = ops0_cm.__enter__()
            ops1_cm = tc.tile_pool(name="ops1", bufs=1, space="PSUM")
            ops1 = ops1_cm.__enter__()
            sps_cm = tc.tile_pool(name="sps", bufs=2, space="PSUM")
            sps = sps_cm.__enter__()

            ot0 = ops0.tile([65, 256 * 4], f32, tag="ot0", name="ot0")
            ot1 = ops1.tile([65, 256 * 4], f32, tag="ot1", name="ot1")

            # pass 0
            prev = None
            for m in range(NCHUNK):
                wt = attn_scores(0, m)
                if prev is not None:
                    attn_ov(0, m - 1, ot0, prev)
                prev = wt
            attn_ov(0, NCHUNK - 1, ot0, prev)
            recip_chain(0, ot0)     # ln/exp on ACT, overlaps pass 1

            # pass 1
            prev = None
            for m in range(NCHUNK):
                wt = attn_scores(1, m)
                if prev is not None:
                    attn_ov(1, m - 1, ot1, prev)
                prev = wt
            attn_ov(1, NCHUNK - 1, ot1, prev)
            recip_chain(1, ot1)

            # tail: scores banks -> recip-broadcast + projection banks
            sps_cm.__exit__(None, None, None)
            rps_cm = tc.tile_pool(name="rps", bufs=1, space="PSUM")
            rps = rps_cm.__enter__()
            pps_cm = tc.tile_pool(name="pps", bufs=1, space="PSUM")
            pps = pps_cm.__enter__()
            pjall = pps.tile([128, 256 * CCHUNK], f32, tag="pj", name="pj")
            pj = [pjall[:, 256 * mm:256 * (mm + 1)] for mm in range(CCHUNK)]

            def bcast(p):
                rbp = rps.tile([64, 1024], f32, tag="rbp", name=f"rbp{p}")
                for j in range(2):
                    nc.tensor.matmul(
                        rbp[:, 512 * j:512 * (j + 1)],
                        ones1[0:1, 0:64],
                        rrb[0:1, 1024 * p + 512 * j:
                            1024 * p + 512 * (j + 1)],
                        start=True, stop=True)
                with nc.allow_low_precision(reason="bf16 rb"):
                    nc.vector.tensor_copy(
                        rbb[:, 1024 * p:1024 * (p + 1)], rbp[:])

            bcast(0)
            norm_mult(0, ot0)
            # first half of projection (pass-0 heads) fills the PE while
            # the pass-1 recip broadcast + normalize complete
            for mm in range(CCHUNK):
                for b in range(4):
                    nc.tensor.matmul(
                        pj[mm],
                        wpb[:, C * b + 128 * mm:C * b + 128 * (mm + 1)],
                        on[:, 256 * b:256 * (b + 1)],
                        start=(b == 0 and mm % 2 == 0), stop=False,
                        skip_group_check=True)
            bcast(1)
            norm_mult(1, ot1)
            with tc.tile_pool(name="psb", bufs=2) as psb:
                # epilogue per bank-pair: one DVE op reads the whole
                # 2-chunk psum bank (residual+bias precombined in xqb)
                for pair in range(2):
                    for mm in (2 * pair, 2 * pair + 1):
                        for b in range(4, HEADS):
                            nc.tensor.matmul(
                                pj[mm],
                                wpb[:, C * b + 128 * mm:
                                    C * b + 128 * (mm + 1)],
                                on[:, 256 * b:256 * (b + 1)],
                                start=False,
                                stop=(b == HEADS - 1 and mm % 2 == 1),
                                skip_group_check=True)
                    outp = psb.tile([128, 2 * NQ], f32, tag="outp",
                                    name=f"outp{pair}")
                    nc.vector.tensor_add(
                        outp[:], pjall[:, 512 * pair:512 * (pair + 1)],
                        xqb[:, 512 * pair:512 * (pair + 1)])
                    for mm in (2 * pair, 2 * pair + 1):
                        nc.sync.dma_start(
                            out_d[128 * mm:128 * (mm + 1), :],
                            outp[:, 256 * (mm - 2 * pair):
                                 256 * (mm - 2 * pair + 1)])
            pps_cm.__exit__(None, None, None)
            rps_cm.__exit__(None, None, None)
            ops1_cm.__exit__(None, None, None)
            ops0_cm.__exit__(None, None, None)

    if split_waits:
        _split_sync_waits(nc)
    return nc


# ---------------------------------------------------------------------------
# host-side input prep + entry point
# ---------------------------------------------------------------------------

def _prep_inputs(x, valid_indices_mask, attendable_indices, gn_w, gn_b,
                 wq_, bq_, wk_, bk_, wv_, bv_, wp_, bp_):
    x = np.asarray(x, np.float32).reshape(C, N)
    idx = np.asarray(attendable_indices, np.int64)
    val = np.asarray(valid_indices_mask, np.float32)
    cnt_qn = np.zeros((N, N), np.float32)       # [q, n]
    rows = np.repeat(np.arange(N), K_IDX)
    np.add.at(cnt_qn, (rows, idx.reshape(-1)), val.reshape(-1))
    cntT = np.ascontiguousarray(cnt_qn.T).astype(ml_dtypes.bfloat16)  # [n, q]

    wq_ = np.asarray(wq_, np.float32)
    wk_ = np.asarray(wk_, np.float32)
    wv_ = np.asarray(wv_, np.float32)
    wp_ = np.asarray(wp_, np.float32)
    # wp column for o-channel (d*HEADS + h); our block order stacks head
    # HB[b] rows d-major at 64*b
    wpT = wp_.T                                    # [cin = d*8+h, cout]
    wpTb = np.empty((C, C), np.float32)
    for b in range(HEADS):
        h = HB[b]
        wpTb[64 * b:64 * (b + 1), :] = wpT[h::HEADS, :]

    gind = np.zeros((C, GROUPS), np.float32)
    gind[np.arange(C), np.arange(C) // GSIZE] = 1.0

    smalls = np.zeros((128, 20), np.float32)
    fields = [np.asarray(bk_, np.float32), np.asarray(bq_, np.float32),
              np.asarray(bp_, np.float32), np.asarray(gn_w, np.float32),
              np.asarray(gn_b, np.float32)]
    for k in range(CCHUNK):
        for f, arr in enumerate(fields):
            smalls[:, 5 * k + f] = arr.reshape(C)[128 * k:128 * (k + 1)]
    gind_all = np.zeros((128, 32 * CCHUNK), np.float32)
    for k in range(CCHUNK):
        gind_all[:, 32 * k:32 * (k + 1)] = gind[128 * k:128 * (k + 1), :]
    common = {
        "x": x.astype(ml_dtypes.bfloat16),
        "wkT": np.ascontiguousarray(wk_.T),
        "wqT": np.ascontiguousarray(wq_.T),
        "wvT": np.ascontiguousarray(wv_.T),
        "wpTb": wpTb.astype(ml_dtypes.bfloat16),
        "smalls": smalls,
        "bvrow": np.asarray(bv_, np.float32).reshape(1, C)
            .astype(ml_dtypes.bfloat16),
        "gind": gind_all,
        "gindT": np.ascontiguousarray(gind.T),
    }
    in_maps = []
    for c in range(N_CORES):
        cols = slice(NQ * c, NQ * (c + 1))
        m = dict(common)
        m["xq"] = np.ascontiguousarray(x[:, cols])
        m["cnt"] = np.ascontiguousarray(cntT[:, cols])
        in_maps.append(m)
    return in_maps


def _enable_profile_hook():
    """Register the axon NTFF hook (this container's antenv lacks it)."""
    import antenv
    if 'antenv.axon_hooks' not in sys.modules:
        mod = types.ModuleType('antenv.axon_hooks')
        mod._hook = None
        mod.set_axon_ntff_profile_hook = lambda h: setattr(mod, '_hook', h)
        mod.get_axon_ntff_profile_hook = lambda: mod._hook
        sys.modules['antenv.axon_hooks'] = mod
        antenv.axon_hooks = mod
    from trn_agent_boot.trn_boot import _ntff_profile_via_ctypes
    sys.modules['antenv.axon_hooks'].set_axon_ntff_profile_hook(
        _ntff_profile_via_ctypes('/opt/axon/libaxon_pjrt.so'))
    import concourse.bass_utils as bu
    bu.upload_artifacts = lambda tmpdir: tmpdir


_CACHE = {}


def _run(inputs, trace=False):
    if "nc" not in _CACHE:
        _CACHE["nc"] = _build()
    nc = _CACHE["nc"]
    in_maps = _prep_inputs(
        inputs['x'], inputs['valid_indices_mask'],
        inputs['attendable_indices'], inputs['gn_w'], inputs['gn_b'],
        inputs['wq'], inputs['bq'], inputs['wk'], inputs['bk'],
        inputs['wv'], inputs['bv'], inputs['wp'], inputs['bp'])
    if trace:
        _enable_profile_hook()
    res = run_bass_kernel_spmd(nc, in_maps, list(range(N_CORES)), trace=trace)
    out = np.concatenate([res.results[c]["out"] for c in range(N_CORES)],
                         axis=1).reshape(1, C, N).astype(np.float32)
    return out, res


def kernel(**inputs):
    out, _ = _run(inputs, trace=False)
    return out


# revision 44
# speedup vs baseline: 1.0498x; 1.0498x over previous
"""Trainium2 Bass kernel for nn_AttnBlock_16887811407979 (sparse attention).

Strategy: 8-way sequence-parallel SPMD (each core handles a 256-query
slice, all heads), no collectives. The sparse gather is densified: the
host converts (attendable_indices, valid_indices_mask) into a dense
count matrix C[n, q], so softmax-over-slots == count-weighted dense
softmax: W[n,q] = C[n,q]*exp(S^T[n,q]); O[q] = (W^T V)/sum_n W[n,q].

v4:
  * all big tensors pre-tiled on the host into SBUF layout so every DMA
    is a contiguous 2D copy with large descriptors; everything moves on
    the two HWDGE rings in priority order (x -> xq -> wk -> wq -> wv ->
    cnt -> wpb); x gets full aggregate bandwidth.
  * conv burst interleaved with attention chunks: exp on ACT (~1.34us
    per [128,1024], the attention bottleneck) hides under conv matmuls.
  * GroupNorm affine by column group so kconv(0) starts right after
    stats; score path in f32r (x ships bf16), V/attn-weights bf16.
  * softmax 1/rowsums = exp(-ln(s*2^-32)) - 32ln2) on ACT (Ln table
    clips near 2^64); PSUM: ot0 2 + ot1 2 + scores 2 + conv 2 banks.
"""
import sys
import types
import contextlib

sys.path.insert(0, '/opt/trn_rl_repo')
sys.path.insert(0, '/root/.axon_site')

import numpy as np
import ml_dtypes

import concourse.bass as bass
import concourse.tile as tile
from concourse import mybir
from concourse.vector_clock import ScopedClock
from concourse.bass_utils import run_bass_kernel_spmd

f32 = mybir.dt.float32
f32r = mybir.dt.float32r
bf16 = mybir.dt.bfloat16
AF = mybir.ActivationFunctionType
AX = mybir.AxisListType
ALU = mybir.AluOpType

N_CORES = 8
C = 512
N = 2048
HEADS = 8
D = 64
K_IDX = 128
GROUPS = 32
GSIZE = C // GROUPS          # 16 channels per group
NQ = N // N_CORES            # 256 queries per core
NCHUNK = N // 128            # 16 key chunks
CCHUNK = C // 128            # 4 channel chunks
EPS = 1e-6
DUAL_PSUM_MULT = False       # BIR verifier rejects dual-PSUM TensorTensor

# Attention runs in two passes of 4 heads. global block b = 4*pass + lb:
BLK = [4 * (h // 4) + (h % 4) // 2 + 2 * (h % 2) for h in range(HEADS)]
HB = [0] * 8
for _h in range(HEADS):
    HB[BLK[_h]] = _h                                     # b -> h

# ---------------------------------------------------------------------------
# walrus workaround: this container's walrus accepts at most ONE embedded
# sync-wait per engine instruction. Split Tile's multi-wait instructions
# into chains of single-wait NoOps, and do the same for the kernel-tail
# drain that Tile emits at TileContext exit.
# ---------------------------------------------------------------------------
_wsplit = [0]


def _drain_and_barrier_split(self, tick_clock, wait_clock):
    nc = self.nc
    carrier = nc.sync.nop(nofuse=True)
    wait_clock.add_sem_waits(
        carrier.ins, ScopedClock({None: tick_clock.global_clock}))
    si = carrier.ins.sync_info
    waits = list(si.on_wait or []) if si is not None else []
    if len(waits) > 1:
        carrier.ins.sync_info = mybir.SyncInfo(
            on_wait=waits[:1], on_update=list(si.on_update or []))
        for w in waits[1:]:
            extra = nc.sync.nop(nofuse=True)
            extra.ins.sync_info = mybir.SyncInfo(on_wait=[w], on_update=[])
    nc.sync.drain()
    nc.all_engine_barrier(sem_only=True)
    assert self.sems is not None
    popped = nc._tile_sem_poison_stack.pop()
    assert popped is self._sem_poison
    nc.clear_and_free_semaphores(list(self.sems.allocated().values()))
    nc.all_engine_barrier(sem_only=True)


def _split_sync_waits(nc, max_waits=1):
    for f in nc.m.functions:
        for bb in f.blocks:
            insts = bb.instructions
            out = []
            changed = False
            for inst in insts:
                si = inst.sync_info
                waits = list(si.on_wait or []) if si is not None else []
                if len(waits) > max_waits:
                    changed = True
                    for i in range(len(waits) - max_waits):
                        _wsplit[0] += 1
                        nop = mybir.InstNoOp(
                            name=f"I-wsplit-{_wsplit[0]}", ins=[], outs=[])
                        nop.engine = inst.engine
                        nop.sync_info = mybir.SyncInfo(
                            on_wait=[waits[i]], on_update=[])
                        out.append(nop)
                    inst.sync_info = mybir.SyncInfo(
                        on_wait=waits[len(waits) - max_waits:],
                        on_update=list(si.on_update or []))
                out.append(inst)
            if changed:
                if isinstance(insts, list):
                    insts[:] = out
                else:
                    bb.instructions = out


tile.TileContext._drain_and_barrier = _drain_and_barrier_split


# ---------------------------------------------------------------------------
# kernel builder
# ---------------------------------------------------------------------------

def _build(split_waits=True):
    nc = bass.Bass("TRN2", target_bir_lowering=False, debug=False)

    def din(name, shape, dt=f32):
        return nc.dram_tensor(name, shape, dt, kind="ExternalInput").ap()

    # all pre-tiled on host into SBUF layouts
    x_d = din("x", [C, N], bf16)
    xq_d = din("xq", [128, NQ * CCHUNK])
    cnt_d = din("cnt", [128, 256 * NCHUNK], bf16)
    wk_d = din("wk", [128, C * CCHUNK], f32)
    wq_d = din("wq", [128, C * CCHUNK], f32)
    wv_d = din("wv", [128, C * CCHUNK], f32)
    wp_d = din("wp", [64, C * HEADS], bf16)
    smalls_d = din("smalls", [128, 20])
    bvrow_d = din("bvrow", [1, C], bf16)
    gind_d = din("gind", [128, 32 * CCHUNK])
    gindT_d = din("gindT", [GROUPS, C])
    out_d = nc.dram_tensor("out", [C, NQ], f32, kind="ExternalOutput").ap()

    with tile.TileContext(nc) as tc, contextlib.ExitStack() as ctx:
        P = ctx.enter_context(tc.tile_pool(name="persist", bufs=1))
        A = ctx.enter_context(tc.tile_pool(name="phase_a", bufs=1))

        # ---- DMAs in priority order on both HWDGE rings ----
        xt = [A.tile([128, N], bf16, tag=f"xt{k}", name=f"xt{k}")
              for k in range(CCHUNK)]
        for k in range(CCHUNK):
            nc.sync.dma_start(xt[k][:, 0:1024],
                              x_d[128 * k:128 * (k + 1), 0:1024])
            nc.scalar.dma_start(xt[k][:, 1024:2048],
                                x_d[128 * k:128 * (k + 1), 1024:2048])
        smallst = P.tile([128, 20], f32, tag="smalls", name="smalls")
        nc.sync.dma_start(smallst[:], smalls_d)
        gindt = P.tile([128, 32 * CCHUNK], f32, tag="gind", name="gind")
        nc.sync.dma_start(gindt[:], gind_d)
        gindTt = P.tile([GROUPS, C], f32, tag="gindT", name="gindT")
        nc.scalar.dma_start(gindTt[:], gindT_d)
        bvrow_t = P.tile([1, C], bf16, tag="bvrow", name="bvrow")
        nc.scalar.dma_start(bvrow_t[:], bvrow_d)
        xqt = P.tile([128, NQ * CCHUNK], f32, tag="xqt", name="xqt")
        nc.sync.dma_start(xqt[:, 0:512], xq_d[:, 0:512])
        nc.scalar.dma_start(xqt[:, 512:1024], xq_d[:, 512:1024])

        def wload(wt, dram, half):
            nc.sync.dma_start(wt[:, 0:half], dram[:, 0:half])
            nc.scalar.dma_start(wt[:, half:2 * half], dram[:, half:2 * half])

        cntt = P.tile([128, 256 * NCHUNK], bf16, tag="cntt", name="cntt")
        wload(cntt, cnt_d, 2048)
        wpb = P.tile([64, C * HEADS], bf16, tag="wpb", name="wpb")
        wload(wpb, wp_d, 2048)

        # f32->f32r cast DMAs must go through the gpsimd SWDGE; gate
        # them behind the last x chunk so x keeps full DMA bandwidth.
        gate = P.tile([32, 1], bf16, tag="gate", name="gate")
        nc.gpsimd.tensor_copy(gate[:], xt[CCHUNK - 1][96:128, 2047:2048])
        wkt = P.tile([128, C * CCHUNK], f32r, tag="wkt", name="wkt")
        nc.gpsimd.dma_start(wkt[:], wk_d)
        wqt = P.tile([128, C * CCHUNK], f32r, tag="wqt", name="wqt")
        nc.gpsimd.dma_start(wqt[:], wq_d)
        wvt = P.tile([128, C * CCHUNK], f32r, tag="wvt", name="wvt")
        nc.gpsimd.dma_start(wvt[:], wv_d)

        h32 = [P.tile([128, N], f32r, tag=f"h32{k}", name=f"h32{k}")
               for k in range(CCHUNK)]
        hq32 = P.tile([128, NQ * CCHUNK], f32r, tag="hq32", name="hq32")

        def sm(k, f):
            return smallst[:, 5 * k + f:5 * k + f + 1]

        # ---- GroupNorm stats -> per-channel scale A / bias B ----
        s1h = [[P.tile([128, 1], f32, tag=f"s1h{k}_{j}", name=f"s1h{k}_{j}")
                for j in range(2)] for k in range(CCHUNK)]
        s2h = [[P.tile([128, 1], f32, tag=f"s2h{k}_{j}", name=f"s2h{k}_{j}")
                for j in range(2)] for k in range(CCHUNK)]
        s12 = [P.tile([128, 2], f32, tag=f"s12{k}", name=f"s12{k}")
               for k in range(CCHUNK)]
        At = [P.tile([128, 1], f32, tag=f"A{k}", name=f"A{k}")
              for k in range(CCHUNK)]
        Bt = [P.tile([128, 1], f32, tag=f"B{k}", name=f"B{k}")
              for k in range(CCHUNK)]
        with tc.tile_pool(name="gnps", bufs=1, space="PSUM") as gnps:
            for k in range(CCHUNK):
                for j in range(2):
                    half = xt[k][:, 1024 * j:1024 * (j + 1)]
                    nc.vector.tensor_reduce(s1h[k][j][:], half,
                                            axis=AX.X, op=ALU.add)
                    sq = A.tile([128, 1024], bf16, tag="sq", name="sq")
                    nc.scalar.activation(sq[:], half, AF.Square,
                                         accum_out=s2h[k][j][:])
                nc.vector.tensor_add(s12[k][:, 0:1], s1h[k][0][:],
                                     s1h[k][1][:])
                nc.vector.tensor_add(s12[k][:, 1:2], s2h[k][0][:],
                                     s2h[k][1][:])
            gs = gnps.tile([GROUPS, 2], f32, tag="gs", name="gs")
            for k in range(CCHUNK):
                for c2 in range(2):
                    nc.tensor.matmul(gs[:, c2:c2 + 1],
                                     gindt[:, 32 * k:32 * (k + 1)],
                                     s12[k][:, c2:c2 + 1],
                                     start=(k == 0 and c2 == 0),
                                     stop=(k == CCHUNK - 1 and c2 == 1),
                                     skip_group_check=True)
            mstat = P.tile([GROUPS, 2], f32, tag="mstat", name="mstat")
            inv_n = 1.0 / (GSIZE * N)
            nc.vector.tensor_scalar_mul(mstat[:, 0:1], gs[:, 0:1], inv_n)
            msq = P.tile([GROUPS, 1], f32, tag="msq", name="msq")
            nc.vector.tensor_scalar_mul(msq[:], gs[:, 1:2], inv_n)
            m2 = P.tile([GROUPS, 1], f32, tag="m2", name="m2")
            nc.vector.tensor_mul(m2[:], mstat[:, 0:1], mstat[:, 0:1])
            var = P.tile([GROUPS, 1], f32, tag="var", name="var")
            nc.vector.tensor_sub(var[:], msq[:], m2[:])
            nc.vector.tensor_scalar_add(var[:], var[:], float(EPS))
            std = P.tile([GROUPS, 1], f32, tag="std", name="std")
            nc.scalar.activation(std[:], var[:], AF.Sqrt)
            nc.vector.reciprocal(mstat[:, 1:2], std[:])
            mr = [P.tile([128, 2], f32, tag=f"mr{k}", name=f"mr{k}")
                  for k in range(CCHUNK)]
            for k in range(CCHUNK):
                mrp = gnps.tile([128, 2], f32, tag="mrp", name="mrp", bufs=2)
                nc.tensor.matmul(mrp[:], gindTt[:, 128 * k:128 * (k + 1)],
                                 mstat[:], start=True, stop=True)
                nc.vector.tensor_copy(mr[k][:], mrp[:])
            for k in range(CCHUNK):
                nc.vector.tensor_mul(At[k][:], sm(k, 3), mr[k][:, 1:2])
                tmp = P.tile([128, 1], f32, tag="tmpB", name="tmpB")
                nc.vector.tensor_mul(tmp[:], mr[k][:, 0:1], At[k][:])
                nc.vector.tensor_sub(Bt[k][:], sm(k, 4), tmp[:])
            # hq affine first (gates qconv), then h by column group so
            # kconv(0) can start after the first group (ACT/DVE split)
            for k in range(CCHUNK):
                nc.scalar.activation(hq32[:, NQ * k:NQ * (k + 1)],
                                     xqt[:, NQ * k:NQ * (k + 1)],
                                     AF.Identity,
                                     bias=Bt[k][:, 0:1], scale=At[k][:, 0:1])
            for j in range(4):
                for k in range(CCHUNK):
                    cols = slice(512 * j, 512 * (j + 1))
                    if (j + k) % 2 == 0:
                        nc.scalar.activation(h32[k][:, cols], xt[k][:, cols],
                                             AF.Identity, bias=Bt[k][:, 0:1],
                                             scale=At[k][:, 0:1])
                    else:
                        with nc.allow_low_precision(reason="f32r affine"):
                            nc.vector.tensor_scalar(
                                h32[k][:, cols], xt[k][:, cols],
                                At[k][:, 0:1], Bt[k][:, 0:1],
                                op0=ALU.mult, op1=ALU.add)

        # residual + proj bias, precombined for the tail epilogue
        xqb = P.tile([128, NQ * CCHUNK], f32, tag="xqb", name="xqb")
        for k in range(CCHUNK):
            nc.scalar.activation(xqb[:, NQ * k:NQ * (k + 1)],
                                 xqt[:, NQ * k:NQ * (k + 1)],
                                 AF.Identity, bias=sm(k, 2))

        kt = [P.tile([128, N], f32r, tag=f"kt{k}", name=f"kt{k}")
              for k in range(CCHUNK)]
        qt = [P.tile([128, NQ], f32r, tag=f"qt{k}", name=f"qt{k}")
              for k in range(CCHUNK)]
        vt = [P.tile([128, 65 * HEADS], bf16, tag=f"vt{m}", name=f"vt{m}")
              for m in range(NCHUNK)]
        on = P.tile([64, 256 * HEADS], bf16, tag="on", name="on")
        tln = P.tile([1, 256 * HEADS], f32, tag="tln", name="tln")
        rrb = P.tile([1, 256 * HEADS], bf16, tag="rrb", name="rrb")
        rbb = P.tile([64, 256 * HEADS], bf16, tag="rbb", name="rbb")
        ones1 = P.tile([1, 128], bf16, tag="ones1", name="ones1")
        nc.vector.memset(ones1[:], 1.0)
        LN2_32 = float(32 * np.log(2.0))
        bias_ln = P.tile([1, 1], f32, tag="bias_ln", name="bias_ln")
        nc.vector.memset(bias_ln[:], -LN2_32)

        def recip_chain(p, ot):
            # 1/rowsums: exp(-ln(s*2^-32) - 32ln2) on ACT. The 2^-32
            # scale keeps the Ln table input under its ~2^64 clip
            # (rowsums reach ~e^52).
            sl = slice(1024 * p, 1024 * (p + 1))
            nc.scalar.activation(tln[0:1, sl], ot[64:65, :], AF.Ln,
                                 scale=float(2.0 ** -32))
            nc.scalar.activation(rrb[0:1, sl], tln[0:1, sl], AF.Exp,
                                 scale=-1.0, bias=bias_ln[:, 0:1])

        # ---- fused conv + attention phase ----
        with tc.tile_pool(name="asb", bufs=3) as asb:
            ops0_cm = tc.tile_pool(name="ops0", bufs=1, space="PSUM")
            ops0 = ops0_cm.__enter__()
            ops1_cm = tc.tile_pool(name="ops1", bufs=1, space="PSUM")
            ops1 = ops1_cm.__enter__()
            sps_cm = tc.tile_pool(name="sps", bufs=1, space="PSUM")
            sps = sps_cm.__enter__()
            cps_cm = tc.tile_pool(name="cps", bufs=2, space="PSUM")
            cps = cps_cm.__enter__()

            ot0 = ops0.tile([65, 256 * 4], f32, tag="ot0", name="ot0")
            ot1 = ops1.tile([65, 256 * 4], f32, tag="ot1", name="ot1")

            def kchain(j, m):
                cols = slice(512 * j, 512 * (j + 1))
                pk = cps.tile([128, 512], f32, tag="cp", name="cpk")
                for ci in range(CCHUNK):
                    nc.tensor.matmul(
                        pk[:],
                        wkt[:, C * ci + 128 * m:C * ci + 128 * (m + 1)],
                        h32[ci][:, cols],
                        start=(ci == 0), stop=(ci == CCHUNK - 1))
                with nc.allow_low_precision(reason="f32r k"):
                    if m % 2 == 0:
                        nc.scalar.activation(kt[m][:, cols], pk[:],
                                             AF.Identity, bias=sm(m, 0))
                    else:
                        nc.vector.tensor_scalar_add(kt[m][:, cols],
                                                    pk[:], sm(m, 0))

            def kconv_colgroup(j):
                for m in range(CCHUNK):
                    kchain(j, m)

            def qconv():
                for m in range(CCHUNK):
                    pq = cps.tile([128, 512], f32, tag="cp",
                                  name="cpq")[:, 0:NQ]
                    for ci in range(CCHUNK):
                        nc.tensor.matmul(
                            pq[:],
                            wqt[:, C * ci + 128 * m:C * ci + 128 * (m + 1)],
                            hq32[:, NQ * ci:NQ * (ci + 1)],
                            start=(ci == 0), stop=(ci == CCHUNK - 1))
                    with nc.allow_low_precision(reason="f32r q"):
                        nc.scalar.activation(qt[m][:], pq[:], AF.Identity,
                                             bias=sm(m, 1))

            def vconv(m):
                pv = cps.tile([128, C], f32, tag="cp", name="cpv")
                for ci in range(CCHUNK):
                    nc.tensor.matmul(pv[:],
                                     h32[ci][:, 128 * m:128 * (m + 1)],
                                     wvt[:, C * ci:C * (ci + 1)],
                                     start=(ci == 0),
                                     stop=(ci == CCHUNK - 1))
                dst = vt[m][:].rearrange("p (h e) -> p h e",
                                         h=HEADS)[:, :, 0:64]
                nc.vector.scalar_tensor_tensor(
                    dst, pv[:].rearrange("p (h d) -> p h d", h=HEADS), 1.0,
                    bvb[:].rearrange("p (h d) -> p h d", h=HEADS),
                    op0=ALU.mult, op1=ALU.add)
                ones_cols = vt[m][:].rearrange(
                    "p (h e) -> p h e", h=HEADS)[:, :, 64:65]
                nc.gpsimd.memset(ones_cols, 1.0)

            def attn_scores(p, m):
                st = sps.tile([128, 1024], f32, tag="st", name=f"st{p}_{m}")
                for h in range(4 * p, 4 * p + 4):
                    par = h % 2
                    cm = h // 2
                    lb = BLK[h] - 4 * p
                    nc.tensor.matmul(
                        st[:, 256 * lb:256 * (lb + 1)],
                        kt[cm][64 * par:64 * (par + 1),
                               128 * m:128 * (m + 1)],
                        qt[cm][64 * par:64 * (par + 1), :],
                        start=True, stop=True)
                et = asb.tile([128, 1024], bf16, tag="et", name=f"et{p}_{m}")
                nc.scalar.activation(et[:], st[:], AF.Exp)
                wt = asb.tile([128, 1024], bf16, tag="wt", name=f"wt{p}_{m}")
                nc.vector.tensor_mul(
                    wt[:].rearrange("p (b q) -> p b q", b=4),
                    et[:].rearrange("p (b q) -> p b q", b=4),
                    cntt[:, 256 * m:256 * (m + 1)].unsqueeze(1)
                        .broadcast_to([128, 4, NQ]))
                return wt

            ots = [ot0, ot1]

            def attn_ov(p, m, wt):
                for h in range(4 * p, 4 * p + 4):
                    lb = BLK[h] - 4 * p
                    nc.tensor.matmul(
                        ots[p][0:65, 256 * lb:256 * (lb + 1)],
                        vt[m][:, 65 * h:65 * h + 65],
                        wt[:, 256 * lb:256 * (lb + 1)],
                        start=(m == 0 and lb % 2 == 0),
                        stop=(m == NCHUNK - 1 and lb % 2 == 1),
                        skip_group_check=True)

            # interleaved conv + attention, pipelined across groups:
            # group j's attention pieces alternate 1:1 with group j+1's
            # conv chains, so the PE always has conv work while ACT
            # chews the exps (1.34us each, the attention bottleneck).
            bvb = P.tile([128, C], f32, tag="bvb", name="bvb")
            prevs = [None, None]
            pm = [None, None]

            def attn_piece(p, m):
                wt = attn_scores(p, m)
                if prevs[p] is not None:
                    attn_ov(p, pm[p], prevs[p])
                prevs[p] = wt
                pm[p] = m

            # lead-in: group 0 convs + Q conv + bv broadcast
            kconv_colgroup(0)
            qconv()
            pbv2 = cps.tile([128, C], f32, tag="cp", name="cpbv2")
            nc.tensor.matmul(pbv2[:], ones1[:], bvrow_t[:],
                             start=True, stop=True)
            nc.vector.tensor_copy(bvb[:], pbv2[:])
            for m in range(4):
                vconv(m)
            for j in range(4):
                conv_pieces = []
                if j < 3:
                    conv_pieces = (
                        [(lambda jj, mm: (lambda: kchain(jj, mm)))(j + 1, m)
                         for m in range(CCHUNK)] +
                        [(lambda mm: (lambda: vconv(mm)))(4 * (j + 1) + i)
                         for i in range(4)])
                attn_pieces = [(0, 4 * j + i) for i in range(4)] + \
                              [(1, 4 * j + i) for i in range(4)]
                for i, (p, m) in enumerate(attn_pieces):
                    if i < len(conv_pieces):
                        conv_pieces[i]()
                    attn_piece(p, m)
            attn_ov(0, NCHUNK - 1, prevs[0])
            recip_chain(0, ot0)
            attn_ov(1, NCHUNK - 1, prevs[1])
            recip_chain(1, ot1)

            # tail: conv+score banks -> recip-broadcast + projection
            cps_cm.__exit__(None, None, None)
            sps_cm.__exit__(None, None, None)
            rps_cm = tc.tile_pool(name="rps", bufs=1, space="PSUM")
            rps = rps_cm.__enter__()
            pjall = rps.tile([128, 256 * CCHUNK], f32, tag="pj", name="pj")
            pj = [pjall[:, 256 * mm:256 * (mm + 1)] for mm in range(CCHUNK)]
            rbp = {}

            def bcast(p):
                rbp[p] = rps.tile([64, 1024], f32, tag="rbp",
                                  name=f"rbp{p}")
                for jj in range(2):
                    nc.tensor.matmul(
                        rbp[p][:, 512 * jj:512 * (jj + 1)],
                        ones1[0:1, 0:64],
                        rrb[0:1, 1024 * p + 512 * jj:
                            1024 * p + 512 * (jj + 1)],
                        start=True, stop=True)

            def norm_mult(p, ot):
                sl = slice(1024 * p, 1024 * (p + 1))
                with nc.allow_low_precision(reason="bf16 on"):
                    if DUAL_PSUM_MULT:
                        nc.vector.tensor_mul(on[:, sl], ot[0:64, :],
                                             rbp[p][:])
                    else:
                        nc.vector.tensor_copy(rbb[:, sl], rbp[p][:])
                        nc.vector.tensor_mul(on[:, sl], ot[0:64, :],
                                             rbb[:, sl])

            bcast(0)
            norm_mult(0, ot0)
            for mm in range(CCHUNK):
                for b in range(4):
                    nc.tensor.matmul(
                        pj[mm],
                        wpb[:, C * b + 128 * mm:C * b + 128 * (mm + 1)],
                        on[:, 256 * b:256 * (b + 1)],
                        start=(b == 0 and mm % 2 == 0), stop=False,
                        skip_group_check=True)
            bcast(1)
            norm_mult(1, ot1)
            with tc.tile_pool(name="psb", bufs=2) as psb:
                # epilogue per bank-pair: one DVE op reads the whole
                # 2-chunk psum bank (residual+bias precombined in xqb)
                for pair in range(2):
                    for mm in (2 * pair, 2 * pair + 1):
                        for b in range(4, HEADS):
                            nc.tensor.matmul(
                                pj[mm],
                                wpb[:, C * b + 128 * mm:
                                    C * b + 128 * (mm + 1)],
                                on[:, 256 * b:256 * (b + 1)],
                                start=False,
                                stop=(b == HEADS - 1 and mm % 2 == 1),
                                skip_group_check=True)
                    outp = psb.tile([128, 2 * NQ], f32, tag="outp",
                                    name=f"outp{pair}")
                    nc.vector.tensor_add(
                        outp[:], pjall[:, 512 * pair:512 * (pair + 1)],
                        xqb[:, 512 * pair:512 * (pair + 1)])
                    for mm in (2 * pair, 2 * pair + 1):
                        nc.sync.dma_start(
                            out_d[128 * mm:128 * (mm + 1), :],
                            outp[:, 256 * (mm - 2 * pair):
                                 256 * (mm - 2 * pair + 1)])
            rps_cm.__exit__(None, None, None)
            ops1_cm.__exit__(None, None, None)
            ops0_cm.__exit__(None, None, None)

    if split_waits:
        _split_sync_waits(nc)
    return nc


# ---------------------------------------------------------------------------
# host-side input prep + entry point
# ---------------------------------------------------------------------------

def _tile_rows(a, p):
    """[K*p, F] -> [p, K*F] with row-chunk k at cols [k*F:(k+1)*F]."""
    kk = a.shape[0] // p
    return np.ascontiguousarray(
        a.reshape(kk, p, a.shape[1]).transpose(1, 0, 2).reshape(
            p, kk * a.shape[1]))


def _prep_inputs(x, valid_indices_mask, attendable_indices, gn_w, gn_b,
                 wq_, bq_, wk_, bk_, wv_, bv_, wp_, bp_):
    x = np.asarray(x, np.float32).reshape(C, N)
    idx = np.asarray(attendable_indices, np.int64)
    val = np.asarray(valid_indices_mask, np.float32)
    cnt_qn = np.zeros((N, N), np.float32)       # [q, n]
    rows = np.repeat(np.arange(N), K_IDX)
    np.add.at(cnt_qn, (rows, idx.reshape(-1)), val.reshape(-1))
    cntT = np.ascontiguousarray(cnt_qn.T).astype(ml_dtypes.bfloat16)  # [n, q]

    wq_ = np.asarray(wq_, np.float32)
    wk_ = np.asarray(wk_, np.float32)
    wv_ = np.asarray(wv_, np.float32)
    wp_ = np.asarray(wp_, np.float32)
    # wp column for o-channel (d*HEADS + h); our block order stacks head
    # HB[b] rows d-major at 64*b
    wpT = wp_.T                                    # [cin = d*8+h, cout]
    wpTb = np.empty((C, C), np.float32)
    for b in range(HEADS):
        h = HB[b]
        wpTb[64 * b:64 * (b + 1), :] = wpT[h::HEADS, :]

    gind = np.zeros((C, GROUPS), np.float32)
    gind[np.arange(C), np.arange(C) // GSIZE] = 1.0

    smalls = np.zeros((128, 20), np.float32)
    fields = [np.asarray(bk_, np.float32), np.asarray(bq_, np.float32),
              np.asarray(bp_, np.float32), np.asarray(gn_w, np.float32),
              np.asarray(gn_b, np.float32)]
    for k in range(CCHUNK):
        for f, arr in enumerate(fields):
            smalls[:, 5 * k + f] = arr.reshape(C)[128 * k:128 * (k + 1)]
    gind_all = np.zeros((128, 32 * CCHUNK), np.float32)
    for k in range(CCHUNK):
        gind_all[:, 32 * k:32 * (k + 1)] = gind[128 * k:128 * (k + 1), :]
    common = {
        "x": x.astype(ml_dtypes.bfloat16),
        "wk": _tile_rows(np.ascontiguousarray(wk_.T), 128),
        "wq": _tile_rows(np.ascontiguousarray(wq_.T), 128),
        "wv": _tile_rows(np.ascontiguousarray(wv_.T), 128),
        "wp": _tile_rows(wpTb, 64).astype(ml_dtypes.bfloat16),
        "smalls": smalls,
        "bvrow": np.asarray(bv_, np.float32).reshape(1, C)
            .astype(ml_dtypes.bfloat16),
        "gind": gind_all,
        "gindT": np.ascontiguousarray(gind.T),
    }
    in_maps = []
    for c in range(N_CORES):
        cols = slice(NQ * c, NQ * (c + 1))
        m = dict(common)
        m["xq"] = _tile_rows(np.ascontiguousarray(x[:, cols]), 128)
        m["cnt"] = _tile_rows(np.ascontiguousarray(cntT[:, cols]), 128)
        in_maps.append(m)
    return in_maps


def _enable_profile_hook():
    """Register the axon NTFF hook (this container's antenv lacks it)."""
    import antenv
    if 'antenv.axon_hooks' not in sys.modules:
        mod = types.ModuleType('antenv.axon_hooks')
        mod._hook = None
        mod.set_axon_ntff_profile_hook = lambda h: setattr(mod, '_hook', h)
        mod.get_axon_ntff_profile_hook = lambda: mod._hook
        sys.modules['antenv.axon_hooks'] = mod
        antenv.axon_hooks = mod
    from trn_agent_boot.trn_boot import _ntff_profile_via_ctypes
    sys.modules['antenv.axon_hooks'].set_axon_ntff_profile_hook(
        _ntff_profile_via_ctypes('/opt/axon/libaxon_pjrt.so'))
    import concourse.bass_utils as bu
    bu.upload_artifacts = lambda tmpdir: tmpdir


_CACHE = {}


def _run(inputs, trace=False):
    if "nc" not in _CACHE:
        _CACHE["nc"] = _build()
    nc = _CACHE["nc"]
    in_maps = _prep_inputs(
        inputs['x'], inputs['valid_indices_mask'],
        inputs['attendable_indices'], inputs['gn_w'], inputs['gn_b'],
        inputs['wq'], inputs['bq'], inputs['wk'], inputs['bk'],
        inputs['wv'], inputs['bv'], inputs['wp'], inputs['bp'])
    if trace:
        _enable_profile_hook()
    res = run_bass_kernel_spmd(nc, in_maps, list(range(N_CORES)), trace=trace)
    out = np.concatenate([res.results[c]["out"] for c in range(N_CORES)],
                         axis=1).reshape(1, C, N).astype(np.float32)
    return out, res


def kernel(**inputs):
    out, _ = _run(inputs, trace=False)
    return out


# revision 48
# speedup vs baseline: 1.0577x; 1.0075x over previous
"""Trainium2 Bass kernel for nn_AttnBlock_16887811407979 (sparse attention).

Strategy: 8-way sequence-parallel SPMD (each core handles a 256-query
slice, all heads), no collectives. The sparse gather is densified: the
host converts (attendable_indices, valid_indices_mask) into a dense
count matrix C[n, q], so softmax-over-slots == count-weighted dense
softmax: W[n,q] = C[n,q]*exp(S^T[n,q]); O[q] = (W^T V)/sum_n W[n,q].

v4:
  * all big tensors pre-tiled on the host into SBUF layout so every DMA
    is a contiguous 2D copy with large descriptors; everything moves on
    the two HWDGE rings in priority order (x -> xq -> wk -> wq -> wv ->
    cnt -> wpb); x gets full aggregate bandwidth.
  * conv burst interleaved with attention chunks: exp on ACT (~1.34us
    per [128,1024], the attention bottleneck) hides under conv matmuls.
  * GroupNorm affine by column group so kconv(0) starts right after
    stats; score path in f32r (x ships bf16), V/attn-weights bf16.
  * softmax 1/rowsums = exp(-ln(s*2^-32)) - 32ln2) on ACT (Ln table
    clips near 2^64); PSUM: ot0 2 + ot1 2 + scores 2 + conv 2 banks.
"""
import sys
import types
import contextlib

sys.path.insert(0, '/opt/trn_rl_repo')
sys.path.insert(0, '/root/.axon_site')

import numpy as np
import ml_dtypes

import concourse.bass as bass
import concourse.tile as tile
from concourse import mybir
from concourse.vector_clock import ScopedClock
from concourse.bass_utils import run_bass_kernel_spmd

f32 = mybir.dt.float32
f32r = mybir.dt.float32r
bf16 = mybir.dt.bfloat16
AF = mybir.ActivationFunctionType
AX = mybir.AxisListType
ALU = mybir.AluOpType

N_CORES = 8
C = 512
N = 2048
HEADS = 8
D = 64
K_IDX = 128
GROUPS = 32
GSIZE = C // GROUPS          # 16 channels per group
NQ = N // N_CORES            # 256 queries per core
NCHUNK = N // 128            # 16 key chunks
CCHUNK = C // 128            # 4 channel chunks
EPS = 1e-6
DUAL_PSUM_MULT = False       # BIR verifier rejects dual-PSUM TensorTensor

# Attention runs in two passes of 4 heads. global block b = 4*pass + lb:
BLK = [4 * (h // 4) + (h % 4) // 2 + 2 * (h % 2) for h in range(HEADS)]
HB = [0] * 8
for _h in range(HEADS):
    HB[BLK[_h]] = _h                                     # b -> h

# ---------------------------------------------------------------------------
# walrus workaround: this container's walrus accepts at most ONE embedded
# sync-wait per engine instruction. Split Tile's multi-wait instructions
# into chains of single-wait NoOps, and do the same for the kernel-tail
# drain that Tile emits at TileContext exit.
# ---------------------------------------------------------------------------
_wsplit = [0]


def _drain_and_barrier_split(self, tick_clock, wait_clock):
    nc = self.nc
    carrier = nc.sync.nop(nofuse=True)
    wait_clock.add_sem_waits(
        carrier.ins, ScopedClock({None: tick_clock.global_clock}))
    si = carrier.ins.sync_info
    waits = list(si.on_wait or []) if si is not None else []
    if len(waits) > 1:
        carrier.ins.sync_info = mybir.SyncInfo(
            on_wait=waits[:1], on_update=list(si.on_update or []))
        for w in waits[1:]:
            extra = nc.sync.nop(nofuse=True)
            extra.ins.sync_info = mybir.SyncInfo(on_wait=[w], on_update=[])
    nc.sync.drain()
    nc.all_engine_barrier(sem_only=True)
    assert self.sems is not None
    popped = nc._tile_sem_poison_stack.pop()
    assert popped is self._sem_poison
    nc.clear_and_free_semaphores(list(self.sems.allocated().values()))
    nc.all_engine_barrier(sem_only=True)


def _split_sync_waits(nc, max_waits=1):
    for f in nc.m.functions:
        for bb in f.blocks:
            insts = bb.instructions
            out = []
            changed = False
            for inst in insts:
                si = inst.sync_info
                waits = list(si.on_wait or []) if si is not None else []
                if len(waits) > max_waits:
                    changed = True
                    for i in range(len(waits) - max_waits):
                        _wsplit[0] += 1
                        nop = mybir.InstNoOp(
                            name=f"I-wsplit-{_wsplit[0]}", ins=[], outs=[])
                        nop.engine = inst.engine
                        nop.sync_info = mybir.SyncInfo(
                            on_wait=[waits[i]], on_update=[])
                        out.append(nop)
                    inst.sync_info = mybir.SyncInfo(
                        on_wait=waits[len(waits) - max_waits:],
                        on_update=list(si.on_update or []))
                out.append(inst)
            if changed:
                if isinstance(insts, list):
                    insts[:] = out
                else:
                    bb.instructions = out


tile.TileContext._drain_and_barrier = _drain_and_barrier_split


# ---------------------------------------------------------------------------
# kernel builder
# ---------------------------------------------------------------------------

def _build(split_waits=True):
    nc = bass.Bass("TRN2", target_bir_lowering=False, debug=False)

    def din(name, shape, dt=f32):
        return nc.dram_tensor(name, shape, dt, kind="ExternalInput").ap()

    # all pre-tiled on host into SBUF layouts
    x_d = din("x", [C, N], bf16)
    xq_d = din("xq", [128, NQ * CCHUNK])
    cnt_d = din("cnt", [128, 256 * NCHUNK], bf16)
    wk_d = din("wk", [128, C * CCHUNK], f32)
    wq_d = din("wq", [128, C * CCHUNK], f32)
    wv_d = din("wv", [128, C * CCHUNK], f32)
    wp_d = din("wp", [64, C * HEADS], bf16)
    smalls_d = din("smalls", [128, 20])
    bvrow_d = din("bvrow", [1, C], bf16)
    gind_d = din("gind", [128, 32 * CCHUNK])
    gindT_d = din("gindT", [GROUPS, C])
    out_d = nc.dram_tensor("out", [C, NQ], f32, kind="ExternalOutput").ap()

    with tile.TileContext(nc) as tc, contextlib.ExitStack() as ctx:
        P = ctx.enter_context(tc.tile_pool(name="persist", bufs=1))
        A = ctx.enter_context(tc.tile_pool(name="phase_a", bufs=1))

        # ---- DMAs in priority order on both HWDGE rings ----
        # everything on the sync HWDGE ring in priority order; scalar
        # stays free for ACT compute (its DMA issues would block the
        # GroupNorm squares). Weights go via gpsimd SWDGE (f32->f32r
        # cast), gated behind the last x chunk.
        xt = [A.tile([128, N], bf16, tag=f"xt{k}", name=f"xt{k}")
              for k in range(CCHUNK)]
        for k in range(CCHUNK):
            nc.sync.dma_start(xt[k][:], x_d[128 * k:128 * (k + 1), :])
        smallst = P.tile([128, 20], f32, tag="smalls", name="smalls")
        nc.sync.dma_start(smallst[:], smalls_d)
        gindt = P.tile([128, 32 * CCHUNK], f32, tag="gind", name="gind")
        nc.sync.dma_start(gindt[:], gind_d)
        gindTt = P.tile([GROUPS, C], f32, tag="gindT", name="gindT")
        nc.sync.dma_start(gindTt[:], gindT_d)
        bvrow_t = P.tile([1, C], bf16, tag="bvrow", name="bvrow")
        nc.sync.dma_start(bvrow_t[:], bvrow_d)
        xqt = P.tile([128, NQ * CCHUNK], f32, tag="xqt", name="xqt")
        nc.sync.dma_start(xqt[:], xq_d)
        cntt = P.tile([128, 256 * NCHUNK], bf16, tag="cntt", name="cntt")
        nc.sync.dma_start(cntt[:], cnt_d)
        wpb = P.tile([64, C * HEADS], bf16, tag="wpb", name="wpb")
        nc.sync.dma_start(wpb[:], wp_d)

        gate = P.tile([32, 1], bf16, tag="gate", name="gate")
        nc.gpsimd.tensor_copy(gate[:], xt[CCHUNK - 1][96:128, 2047:2048])
        wkt = P.tile([128, C * CCHUNK], f32r, tag="wkt", name="wkt")
        nc.gpsimd.dma_start(wkt[:], wk_d)
        wqt = P.tile([128, C * CCHUNK], f32r, tag="wqt", name="wqt")
        nc.gpsimd.dma_start(wqt[:], wq_d)
        wvt = P.tile([128, C * CCHUNK], f32r, tag="wvt", name="wvt")
        nc.gpsimd.dma_start(wvt[:], wv_d)

        h32 = [P.tile([128, N], f32r, tag=f"h32{k}", name=f"h32{k}")
               for k in range(CCHUNK)]
        hq32 = P.tile([128, NQ * CCHUNK], f32r, tag="hq32", name="hq32")

        def sm(k, f):
            return smallst[:, 5 * k + f:5 * k + f + 1]

        # ---- GroupNorm stats -> per-channel scale A / bias B ----
        s12 = [P.tile([128, 2], f32, tag=f"s12{k}", name=f"s12{k}")
               for k in range(CCHUNK)]
        At = [P.tile([128, 1], f32, tag=f"A{k}", name=f"A{k}")
              for k in range(CCHUNK)]
        Bt = [P.tile([128, 1], f32, tag=f"B{k}", name=f"B{k}")
              for k in range(CCHUNK)]
        with tc.tile_pool(name="gnps", bufs=1, space="PSUM") as gnps:
            # per-chunk sum (DVE reduce) / sum-of-squares (ACT Square
            # with free-axis accumulator), pipelined as chunks land
            for k in range(CCHUNK):
                nc.vector.tensor_reduce(s12[k][:, 0:1], xt[k][:],
                                        axis=AX.X, op=ALU.add)
                sq = A.tile([128, N], bf16, tag="sq", name="sq", bufs=2)
                nc.scalar.activation(sq[:], xt[k][:], AF.Square,
                                     accum_out=s12[k][:, 1:2])
            gs = gnps.tile([GROUPS, 2], f32, tag="gs", name="gs")
            for k in range(CCHUNK):
                for c2 in range(2):
                    nc.tensor.matmul(gs[:, c2:c2 + 1],
                                     gindt[:, 32 * k:32 * (k + 1)],
                                     s12[k][:, c2:c2 + 1],
                                     start=(k == 0 and c2 == 0),
                                     stop=(k == CCHUNK - 1 and c2 == 1),
                                     skip_group_check=True)
            mstat = P.tile([GROUPS, 2], f32, tag="mstat", name="mstat")
            inv_n = 1.0 / (GSIZE * N)
            nc.vector.tensor_scalar_mul(mstat[:, 0:1], gs[:, 0:1], inv_n)
            msq = P.tile([GROUPS, 1], f32, tag="msq", name="msq")
            nc.vector.tensor_scalar_mul(msq[:], gs[:, 1:2], inv_n)
            m2 = P.tile([GROUPS, 1], f32, tag="m2", name="m2")
            nc.vector.tensor_mul(m2[:], mstat[:, 0:1], mstat[:, 0:1])
            var = P.tile([GROUPS, 1], f32, tag="var", name="var")
            nc.vector.tensor_sub(var[:], msq[:], m2[:])
            nc.vector.tensor_scalar_add(var[:], var[:], float(EPS))
            std = P.tile([GROUPS, 1], f32, tag="std", name="std")
            nc.scalar.activation(std[:], var[:], AF.Sqrt)
            nc.vector.reciprocal(mstat[:, 1:2], std[:])
            mr = [P.tile([128, 2], f32, tag=f"mr{k}", name=f"mr{k}")
                  for k in range(CCHUNK)]
            for k in range(CCHUNK):
                mrp = gnps.tile([128, 2], f32, tag="mrp", name="mrp", bufs=2)
                nc.tensor.matmul(mrp[:], gindTt[:, 128 * k:128 * (k + 1)],
                                 mstat[:], start=True, stop=True)
                nc.vector.tensor_copy(mr[k][:], mrp[:])
            for k in range(CCHUNK):
                nc.vector.tensor_mul(At[k][:], sm(k, 3), mr[k][:, 1:2])
                tmp = P.tile([128, 1], f32, tag="tmpB", name="tmpB")
                nc.vector.tensor_mul(tmp[:], mr[k][:, 0:1], At[k][:])
                nc.vector.tensor_sub(Bt[k][:], sm(k, 4), tmp[:])
            # hq affine first (gates qconv), then h by column group so
            # kconv(0) can start after the first group (ACT/DVE split)
            for k in range(CCHUNK):
                nc.scalar.activation(hq32[:, NQ * k:NQ * (k + 1)],
                                     xqt[:, NQ * k:NQ * (k + 1)],
                                     AF.Identity,
                                     bias=Bt[k][:, 0:1], scale=At[k][:, 0:1])
            for j in range(4):
                for k in range(CCHUNK):
                    cols = slice(512 * j, 512 * (j + 1))
                    if (j + k) % 2 == 0:
                        nc.scalar.activation(h32[k][:, cols], xt[k][:, cols],
                                             AF.Identity, bias=Bt[k][:, 0:1],
                                             scale=At[k][:, 0:1])
                    else:
                        with nc.allow_low_precision(reason="f32r affine"):
                            nc.vector.tensor_scalar(
                                h32[k][:, cols], xt[k][:, cols],
                                At[k][:, 0:1], Bt[k][:, 0:1],
                                op0=ALU.mult, op1=ALU.add)

        # residual + proj bias, precombined for the tail epilogue
        xqb = P.tile([128, NQ * CCHUNK], f32, tag="xqb", name="xqb")
        for k in range(CCHUNK):
            nc.scalar.activation(xqb[:, NQ * k:NQ * (k + 1)],
                                 xqt[:, NQ * k:NQ * (k + 1)],
                                 AF.Identity, bias=sm(k, 2))

        kt = [P.tile([128, N], f32r, tag=f"kt{k}", name=f"kt{k}")
              for k in range(CCHUNK)]
        qt = [P.tile([128, NQ], f32r, tag=f"qt{k}", name=f"qt{k}")
              for k in range(CCHUNK)]
        vt = [P.tile([128, 65 * HEADS], bf16, tag=f"vt{m}", name=f"vt{m}")
              for m in range(NCHUNK)]
        on = P.tile([64, 256 * HEADS], bf16, tag="on", name="on")
        tln = P.tile([1, 256 * HEADS], f32, tag="tln", name="tln")
        rrb = P.tile([1, 256 * HEADS], bf16, tag="rrb", name="rrb")
        rbb = P.tile([64, 256 * HEADS], bf16, tag="rbb", name="rbb")
        ones1 = P.tile([1, 128], bf16, tag="ones1", name="ones1")
        nc.vector.memset(ones1[:], 1.0)
        LN2_32 = float(32 * np.log(2.0))
        bias_ln = P.tile([1, 1], f32, tag="bias_ln", name="bias_ln")
        nc.vector.memset(bias_ln[:], -LN2_32)

        def recip_chain(p, ot):
            # 1/rowsums: exp(-ln(s*2^-32) - 32ln2) on ACT. The 2^-32
            # scale keeps the Ln table input under its ~2^64 clip
            # (rowsums reach ~e^52).
            sl = slice(1024 * p, 1024 * (p + 1))
            nc.scalar.activation(tln[0:1, sl], ot[64:65, :], AF.Ln,
                                 scale=float(2.0 ** -32))
            nc.scalar.activation(rrb[0:1, sl], tln[0:1, sl], AF.Exp,
                                 scale=-1.0, bias=bias_ln[:, 0:1])

        # ---- fused conv + attention phase ----
        # One PSUM pool, 8 banks total: ot0 (2) + ot1 (2) + st (2) +
        # cp (2x1). Tail tiles reuse slots by tag (WAR-tracked).
        with tc.tile_pool(name="asb", bufs=3) as asb, \
                tc.tile_pool(name="aps", bufs=1, space="PSUM") as aps:
            ot0 = aps.tile([65, 256 * 4], f32, tag="ot0", name="ot0")
            ot1 = aps.tile([65, 256 * 4], f32, tag="ot1", name="ot1")

            def kchain(j, m):
                cols = slice(512 * j, 512 * (j + 1))
                pk = aps.tile([128, 512], f32, tag="cp", name="cpk", bufs=2)
                for ci in range(CCHUNK):
                    nc.tensor.matmul(
                        pk[:],
                        wkt[:, C * ci + 128 * m:C * ci + 128 * (m + 1)],
                        h32[ci][:, cols],
                        start=(ci == 0), stop=(ci == CCHUNK - 1))
                with nc.allow_low_precision(reason="f32r k"):
                    if m % 2 == 0:
                        nc.scalar.activation(kt[m][:, cols], pk[:],
                                             AF.Identity, bias=sm(m, 0))
                    else:
                        nc.vector.tensor_scalar_add(kt[m][:, cols],
                                                    pk[:], sm(m, 0))

            def qchain(m):
                pq = aps.tile([128, 512], f32, tag="cp", name="cpq",
                              bufs=2)[:, 0:NQ]
                for ci in range(CCHUNK):
                    nc.tensor.matmul(
                        pq[:],
                        wqt[:, C * ci + 128 * m:C * ci + 128 * (m + 1)],
                        hq32[:, NQ * ci:NQ * (ci + 1)],
                        start=(ci == 0), stop=(ci == CCHUNK - 1))
                with nc.allow_low_precision(reason="f32r q"):
                    nc.scalar.activation(qt[m][:], pq[:], AF.Identity,
                                         bias=sm(m, 1))

            def vconv(m):
                pv = aps.tile([128, C], f32, tag="cp", name="cpv", bufs=2)
                for ci in range(CCHUNK):
                    nc.tensor.matmul(pv[:],
                                     h32[ci][:, 128 * m:128 * (m + 1)],
                                     wvt[:, C * ci:C * (ci + 1)],
                                     start=(ci == 0),
                                     stop=(ci == CCHUNK - 1))
                dst = vt[m][:].rearrange("p (h e) -> p h e",
                                         h=HEADS)[:, :, 0:64]
                nc.vector.scalar_tensor_tensor(
                    dst, pv[:].rearrange("p (h d) -> p h d", h=HEADS), 1.0,
                    bvb[:].rearrange("p (h d) -> p h d", h=HEADS),
                    op0=ALU.mult, op1=ALU.add)
                ones_cols = vt[m][:].rearrange(
                    "p (h e) -> p h e", h=HEADS)[:, :, 64:65]
                nc.gpsimd.memset(ones_cols, 1.0)

            def attn_scores(p, m):
                st = aps.tile([128, 1024], f32, tag="st", name=f"st{p}_{m}")
                for h in range(4 * p, 4 * p + 4):
                    par = h % 2
                    cm = h // 2
                    lb = BLK[h] - 4 * p
                    nc.tensor.matmul(
                        st[:, 256 * lb:256 * (lb + 1)],
                        kt[cm][64 * par:64 * (par + 1),
                               128 * m:128 * (m + 1)],
                        qt[cm][64 * par:64 * (par + 1), :],
                        start=True, stop=True)
                et = asb.tile([128, 1024], bf16, tag="et", name=f"et{p}_{m}")
                nc.scalar.activation(et[:], st[:], AF.Exp)
                wt = asb.tile([128, 1024], bf16, tag="wt", name=f"wt{p}_{m}")
                nc.vector.tensor_mul(
                    wt[:].rearrange("p (b q) -> p b q", b=4),
                    et[:].rearrange("p (b q) -> p b q", b=4),
                    cntt[:, 256 * m:256 * (m + 1)].unsqueeze(1)
                        .broadcast_to([128, 4, NQ]))
                return wt

            ots = [ot0, ot1]

            def attn_ov(p, m, wt):
                for h in range(4 * p, 4 * p + 4):
                    lb = BLK[h] - 4 * p
                    nc.tensor.matmul(
                        ots[p][0:65, 256 * lb:256 * (lb + 1)],
                        vt[m][:, 65 * h:65 * h + 65],
                        wt[:, 256 * lb:256 * (lb + 1)],
                        start=(m == 0 and lb % 2 == 0),
                        stop=(m == NCHUNK - 1 and lb % 2 == 1),
                        skip_group_check=True)

            bvb = P.tile([128, C], f32, tag="bvb", name="bvb")
            prevs = [None, None]
            pm = [None, None]

            def attn_piece(p, m):
                wt = attn_scores(p, m)
                if prevs[p] is not None:
                    attn_ov(p, pm[p], prevs[p])
                prevs[p] = wt
                pm[p] = m

            # lead-in: group-0 convs + Q conv + bv broadcast
            for m in range(CCHUNK):
                kchain(0, m)
            for m in range(CCHUNK):
                qchain(m)
            pbv2 = aps.tile([128, C], f32, tag="cp", name="cpbv2", bufs=2)
            nc.tensor.matmul(pbv2[:], ones1[:], bvrow_t[:], start=True,
                             stop=True)
            nc.vector.tensor_copy(bvb[:], pbv2[:])
            for m in range(4):
                vconv(m)

            # weave: pass-0 chunks run a group behind their convs;
            # pass-1 chunks trail further so pass 0 finishes early and
            # its normalization chain overlaps pass 1's tail.
            zones = [
                ([(1, m) for m in range(CCHUNK)] + [(-1, m) for m in (4, 5, 6, 7)],
                 [(0, 0), (0, 1), (0, 2), (0, 3)]),
                ([(2, m) for m in range(CCHUNK)] + [(-1, m) for m in (8, 9, 10, 11)],
                 [(0, 4), (0, 5), (0, 6), (0, 7), (1, 0), (1, 1), (1, 2),
                  (1, 3)]),
                ([(3, m) for m in range(CCHUNK)] + [(-1, m) for m in (12, 13, 14, 15)],
                 [(0, 8), (0, 9), (0, 10), (0, 11), (1, 4), (1, 5), (1, 6),
                  (1, 7)]),
            ]
            for conv_list, attn_list in zones:
                na, nc_ = len(attn_list), len(conv_list)
                ai = 0
                for ci_, (cj, cm_) in enumerate(conv_list):
                    if cj >= 0:
                        kchain(cj, cm_)
                    else:
                        vconv(cm_)
                    while ai * nc_ < (ci_ + 1) * na:
                        p, m = attn_list[ai]
                        attn_piece(p, m)
                        ai += 1
                while ai < na:
                    p, m = attn_list[ai]
                    attn_piece(p, m)
                    ai += 1

            # zone 4: finish pass 0, start pass-0 normalization while
            # pass 1 keeps the ACT busy
            z4 = [(0, 12), (1, 8), (0, 13), (1, 9), (0, 14), (1, 10),
                  (0, 15), (1, 11)]
            for p, m in z4:
                attn_piece(p, m)
            attn_ov(0, NCHUNK - 1, prevs[0])
            with nc.allow_low_precision(reason="bf16 recip"):
                nc.vector.reciprocal(rrb[0:1, 0:1024], ot0[64:65, :])
            # bcast0 into the two cp slots ([64,512] each)
            rb0 = [aps.tile([64, 512], f32, tag="cp", name=f"rb0_{jj}",
                            bufs=2) for jj in range(2)]
            for jj in range(2):
                nc.tensor.matmul(rb0[jj][:], ones1[0:1, 0:64],
                                 rrb[0:1, 512 * jj:512 * (jj + 1)],
                                 start=True, stop=True)
            with nc.allow_low_precision(reason="bf16 rb"):
                for jj in range(2):
                    nc.vector.tensor_copy(rbb[:, 512 * jj:512 * (jj + 1)],
                                          rb0[jj][:])
            with nc.allow_low_precision(reason="bf16 on"):
                nc.vector.tensor_mul(on[:, 0:1024], ot0[0:64, :],
                                     rbb[:, 0:1024])

            # zone 5: pass-1 tail chunks woven with the first half of
            # the projection (pjp pair tiles live in the cp slots)
            pjp = [aps.tile([128, 512], f32, tag="cp", name=f"pjp{pr}",
                            bufs=2) for pr in range(2)]
            pj = [pjp[mm // 2][:, 256 * (mm % 2):256 * (mm % 2 + 1)]
                  for mm in range(CCHUNK)]

            def proj_piece(mm, blo, bhi):
                for b in range(blo, bhi):
                    nc.tensor.matmul(
                        pj[mm],
                        wpb[:, C * b + 128 * mm:C * b + 128 * (mm + 1)],
                        on[:, 256 * b:256 * (b + 1)],
                        start=(b == 0 and mm % 2 == 0),
                        stop=(b == HEADS - 1 and mm % 2 == 1),
                        skip_group_check=True)

            z5 = [(1, 12), (1, 13), (1, 14), (1, 15)]
            for i, (p, m) in enumerate(z5):
                attn_piece(p, m)
                proj_piece(i, 0, 4)
            attn_ov(1, NCHUNK - 1, prevs[1])

            # tail: pass-1 normalization (ln/exp on ACT) + projection
            nc.scalar.activation(tln[0:1, 1024:2048], ot1[64:65, :],
                                 AF.Ln, scale=float(2.0 ** -32))
            nc.scalar.activation(rrb[0:1, 1024:2048], tln[0:1, 1024:2048],
                                 AF.Exp, scale=-1.0, bias=bias_ln[:, 0:1])
            rbp1 = aps.tile([64, 1024], f32, tag="ot0", name="rbp1")
            for jj in range(2):
                nc.tensor.matmul(rbp1[:, 512 * jj:512 * (jj + 1)],
                                 ones1[0:1, 0:64],
                                 rrb[0:1, 1024 + 512 * jj:
                                     1024 + 512 * (jj + 1)],
                                 start=True, stop=True)
            with nc.allow_low_precision(reason="bf16 rb"):
                nc.vector.tensor_copy(rbb[:, 1024:2048], rbp1[:])
            with nc.allow_low_precision(reason="bf16 on"):
                nc.vector.tensor_mul(on[:, 1024:2048], ot1[0:64, :],
                                     rbb[:, 1024:2048])
            with tc.tile_pool(name="psb", bufs=2) as psb:
                for pair in range(2):
                    for mm in (2 * pair, 2 * pair + 1):
                        proj_piece(mm, 4, HEADS)
                    outp = psb.tile([128, 2 * NQ], f32, tag="outp",
                                    name=f"outp{pair}")
                    nc.vector.tensor_add(outp[:], pjp[pair][:],
                                         xqb[:, 512 * pair:512 * (pair + 1)])
                    for mm in (2 * pair, 2 * pair + 1):
                        nc.sync.dma_start(
                            out_d[128 * mm:128 * (mm + 1), :],
                            outp[:, 256 * (mm - 2 * pair):
                                 256 * (mm - 2 * pair + 1)])

    if split_waits:
        _split_sync_waits(nc)
    return nc


# ---------------------------------------------------------------------------
# host-side input prep + entry point
# ---------------------------------------------------------------------------

def _tile_rows(a, p):
    """[K*p, F] -> [p, K*F] with row-chunk k at cols [k*F:(k+1)*F]."""
    kk = a.shape[0] // p
    return np.ascontiguousarray(
        a.reshape(kk, p, a.shape[1]).transpose(1, 0, 2).reshape(
            p, kk * a.shape[1]))


def _prep_inputs(x, valid_indices_mask, attendable_indices, gn_w, gn_b,
                 wq_, bq_, wk_, bk_, wv_, bv_, wp_, bp_):
    x = np.asarray(x, np.float32).reshape(C, N)
    idx = np.asarray(attendable_indices, np.int64)
    val = np.asarray(valid_indices_mask, np.float32)
    cnt_qn = np.zeros((N, N), np.float32)       # [q, n]
    rows = np.repeat(np.arange(N), K_IDX)
    np.add.at(cnt_qn, (rows, idx.reshape(-1)), val.reshape(-1))
    cntT = np.ascontiguousarray(cnt_qn.T).astype(ml_dtypes.bfloat16)  # [n, q]

    wq_ = np.asarray(wq_, np.float32)
    wk_ = np.asarray(wk_, np.float32)
    wv_ = np.asarray(wv_, np.float32)
    wp_ = np.asarray(wp_, np.float32)
    # wp column for o-channel (d*HEADS + h); our block order stacks head
    # HB[b] rows d-major at 64*b
    wpT = wp_.T                                    # [cin = d*8+h, cout]
    wpTb = np.empty((C, C), np.float32)
    for b in range(HEADS):
        h = HB[b]
        wpTb[64 * b:64 * (b + 1), :] = wpT[h::HEADS, :]

    gind = np.zeros((C, GROUPS), np.float32)
    gind[np.arange(C), np.arange(C) // GSIZE] = 1.0

    smalls = np.zeros((128, 20), np.float32)
    fields = [np.asarray(bk_, np.float32), np.asarray(bq_, np.float32),
              np.asarray(bp_, np.float32), np.asarray(gn_w, np.float32),
              np.asarray(gn_b, np.float32)]
    for k in range(CCHUNK):
        for f, arr in enumerate(fields):
            smalls[:, 5 * k + f] = arr.reshape(C)[128 * k:128 * (k + 1)]
    gind_all = np.zeros((128, 32 * CCHUNK), np.float32)
    for k in range(CCHUNK):
        gind_all[:, 32 * k:32 * (k + 1)] = gind[128 * k:128 * (k + 1), :]
    common = {
        "x": x.astype(ml_dtypes.bfloat16),
        "wk": _tile_rows(np.ascontiguousarray(wk_.T), 128),
        "wq": _tile_rows(np.ascontiguousarray(wq_.T), 128),
        "wv": _tile_rows(np.ascontiguousarray(wv_.T), 128),
        "wp": _tile_rows(wpTb, 64).astype(ml_dtypes.bfloat16),
        "smalls": smalls,
        "bvrow": np.asarray(bv_, np.float32).reshape(1, C)
            .astype(ml_dtypes.bfloat16),
        "gind": gind_all,
        "gindT": np.ascontiguousarray(gind.T),
    }
    in_maps = []
    for c in range(N_CORES):
        cols = slice(NQ * c, NQ * (c + 1))
        m = dict(common)
        m["xq"] = _tile_rows(np.ascontiguousarray(x[:, cols]), 128)
        m["cnt"] = _tile_rows(np.ascontiguousarray(cntT[:, cols]), 128)
        in_maps.append(m)
    return in_maps


def _enable_profile_hook():
    """Register the axon NTFF hook (this container's antenv lacks it)."""
    import antenv
    if 'antenv.axon_hooks' not in sys.modules:
        mod = types.ModuleType('antenv.axon_hooks')
        mod._hook = None
        mod.set_axon_ntff_profile_hook = lambda h: setattr(mod, '_hook', h)
        mod.get_axon_ntff_profile_hook = lambda: mod._hook
        sys.modules['antenv.axon_hooks'] = mod
        antenv.axon_hooks = mod
    from trn_agent_boot.trn_boot import _ntff_profile_via_ctypes
    sys.modules['antenv.axon_hooks'].set_axon_ntff_profile_hook(
        _ntff_profile_via_ctypes('/opt/axon/libaxon_pjrt.so'))
    import concourse.bass_utils as bu
    bu.upload_artifacts = lambda tmpdir: tmpdir


_CACHE = {}


def _run(inputs, trace=False):
    if "nc" not in _CACHE:
        _CACHE["nc"] = _build()
    nc = _CACHE["nc"]
    in_maps = _prep_inputs(
        inputs['x'], inputs['valid_indices_mask'],
        inputs['attendable_indices'], inputs['gn_w'], inputs['gn_b'],
        inputs['wq'], inputs['bq'], inputs['wk'], inputs['bk'],
        inputs['wv'], inputs['bv'], inputs['wp'], inputs['bp'])
    if trace:
        _enable_profile_hook()
    res = run_bass_kernel_spmd(nc, in_maps, list(range(N_CORES)), trace=trace)
    out = np.concatenate([res.results[c]["out"] for c in range(N_CORES)],
                         axis=1).reshape(1, C, N).astype(np.float32)
    return out, res


def kernel(**inputs):
    out, _ = _run(inputs, trace=False)
    return out


# revision 50
# speedup vs baseline: 1.2109x; 1.1449x over previous
"""Trainium2 Bass kernel for nn_AttnBlock_16887811407979 (sparse attention).

Strategy: 8-way sequence-parallel SPMD (each core handles a 256-query
slice, all heads), no collectives. The sparse gather is densified: the
host converts (attendable_indices, valid_indices_mask) into a dense
count matrix C[n, q], so softmax-over-slots == count-weighted dense
softmax: W[n,q] = C[n,q]*exp(S^T[n,q]); O[q] = (W^T V)/sum_n W[n,q].

v4:
  * all big tensors pre-tiled on the host into SBUF layout so every DMA
    is a contiguous 2D copy with large descriptors; everything moves on
    the two HWDGE rings in priority order (x -> xq -> wk -> wq -> wv ->
    cnt -> wpb); x gets full aggregate bandwidth.
  * conv burst interleaved with attention chunks: exp on ACT (~1.34us
    per [128,1024], the attention bottleneck) hides under conv matmuls.
  * GroupNorm affine by column group so kconv(0) starts right after
    stats; score path in f32r (x ships bf16), V/attn-weights bf16.
  * softmax 1/rowsums = exp(-ln(s*2^-32)) - 32ln2) on ACT (Ln table
    clips near 2^64); PSUM: ot0 2 + ot1 2 + scores 2 + conv 2 banks.
"""
import sys
import types
import contextlib

sys.path.insert(0, '/opt/trn_rl_repo')
sys.path.insert(0, '/root/.axon_site')

import numpy as np
import ml_dtypes

import concourse.bass as bass
import concourse.tile as tile
from concourse import mybir
from concourse.vector_clock import ScopedClock
from concourse.bass_utils import run_bass_kernel_spmd

f32 = mybir.dt.float32
f32r = mybir.dt.float32r
fp16 = mybir.dt.float16
bf16 = mybir.dt.bfloat16
AF = mybir.ActivationFunctionType
AX = mybir.AxisListType
ALU = mybir.AluOpType

N_CORES = 8
C = 512
N = 2048
HEADS = 8
D = 64
K_IDX = 128
GROUPS = 32
GSIZE = C // GROUPS          # 16 channels per group
NQ = N // N_CORES            # 256 queries per core
NCHUNK = N // 128            # 16 key chunks
CCHUNK = C // 128            # 4 channel chunks
EPS = 1e-6
DUAL_PSUM_MULT = False       # BIR verifier rejects dual-PSUM TensorTensor

# Attention runs in two passes of 4 heads. global block b = 4*pass + lb:
BLK = [4 * (h // 4) + (h % 4) // 2 + 2 * (h % 2) for h in range(HEADS)]
HB = [0] * 8
for _h in range(HEADS):
    HB[BLK[_h]] = _h                                     # b -> h

# ---------------------------------------------------------------------------
# walrus workaround: this container's walrus accepts at most ONE embedded
# sync-wait per engine instruction. Split Tile's multi-wait instructions
# into chains of single-wait NoOps, and do the same for the kernel-tail
# drain that Tile emits at TileContext exit.
# ---------------------------------------------------------------------------
_wsplit = [0]


def _drain_and_barrier_split(self, tick_clock, wait_clock):
    nc = self.nc
    carrier = nc.sync.nop(nofuse=True)
    wait_clock.add_sem_waits(
        carrier.ins, ScopedClock({None: tick_clock.global_clock}))
    si = carrier.ins.sync_info
    waits = list(si.on_wait or []) if si is not None else []
    if len(waits) > 1:
        carrier.ins.sync_info = mybir.SyncInfo(
            on_wait=waits[:1], on_update=list(si.on_update or []))
        for w in waits[1:]:
            extra = nc.sync.nop(nofuse=True)
            extra.ins.sync_info = mybir.SyncInfo(on_wait=[w], on_update=[])
    nc.sync.drain()
    nc.all_engine_barrier(sem_only=True)
    assert self.sems is not None
    popped = nc._tile_sem_poison_stack.pop()
    assert popped is self._sem_poison
    nc.clear_and_free_semaphores(list(self.sems.allocated().values()))
    nc.all_engine_barrier(sem_only=True)


def _split_sync_waits(nc, max_waits=1):
    for f in nc.m.functions:
        for bb in f.blocks:
            insts = bb.instructions
            out = []
            changed = False
            for inst in insts:
                si = inst.sync_info
                waits = list(si.on_wait or []) if si is not None else []
                if len(waits) > max_waits:
                    changed = True
                    for i in range(len(waits) - max_waits):
                        _wsplit[0] += 1
                        nop = mybir.InstNoOp(
                            name=f"I-wsplit-{_wsplit[0]}", ins=[], outs=[])
                        nop.engine = inst.engine
                        nop.sync_info = mybir.SyncInfo(
                            on_wait=[waits[i]], on_update=[])
                        out.append(nop)
                    inst.sync_info = mybir.SyncInfo(
                        on_wait=waits[len(waits) - max_waits:],
                        on_update=list(si.on_update or []))
                out.append(inst)
            if changed:
                if isinstance(insts, list):
                    insts[:] = out
                else:
                    bb.instructions = out


tile.TileContext._drain_and_barrier = _drain_and_barrier_split


# ---------------------------------------------------------------------------
# kernel builder
# ---------------------------------------------------------------------------

def _build(split_waits=True):
    nc = bass.Bass("TRN2", target_bir_lowering=False, debug=False)

    def din(name, shape, dt=f32):
        return nc.dram_tensor(name, shape, dt, kind="ExternalInput").ap()

    # all pre-tiled on host into SBUF layouts
    x_d = din("x", [C, N], bf16)
    xq_d = din("xq", [128, NQ * CCHUNK])
    cnt_d = din("cnt", [128, 256 * NCHUNK], bf16)
    wk_d = din("wk", [128, C * CCHUNK], fp16)
    wq_d = din("wq", [128, C * CCHUNK], fp16)
    wv_d = din("wv", [128, C * CCHUNK], fp16)
    wp_d = din("wp", [64, C * HEADS], bf16)
    smalls_d = din("smalls", [128, 20])
    bvrow_d = din("bvrow", [1, C], bf16)
    gind_d = din("gind", [128, 32 * CCHUNK])
    gindT_d = din("gindT", [GROUPS, C])
    out_d = nc.dram_tensor("out", [C, NQ], f32, kind="ExternalOutput").ap()

    with tile.TileContext(nc) as tc, contextlib.ExitStack() as ctx:
        P = ctx.enter_context(tc.tile_pool(name="persist", bufs=1))
        A = ctx.enter_context(tc.tile_pool(name="phase_a", bufs=1))

        # ---- DMAs in priority order on both HWDGE rings ----
        # everything on the sync HWDGE ring in priority order; scalar
        # stays free for ACT compute (its DMA issues would block the
        # GroupNorm squares). Weights go via gpsimd SWDGE (f32->f32r
        # cast), gated behind the last x chunk.
        xt = [A.tile([128, N], bf16, tag=f"xt{k}", name=f"xt{k}")
              for k in range(CCHUNK)]
        for k in range(CCHUNK):
            nc.sync.dma_start(xt[k][:], x_d[128 * k:128 * (k + 1), :])
        smallst = P.tile([128, 20], f32, tag="smalls", name="smalls")
        nc.sync.dma_start(smallst[:], smalls_d)
        gindt = P.tile([128, 32 * CCHUNK], f32, tag="gind", name="gind")
        nc.sync.dma_start(gindt[:], gind_d)
        gindTt = P.tile([GROUPS, C], f32, tag="gindT", name="gindT")
        nc.sync.dma_start(gindTt[:], gindT_d)
        bvrow_t = P.tile([1, C], bf16, tag="bvrow", name="bvrow")
        nc.sync.dma_start(bvrow_t[:], bvrow_d)
        xqt = P.tile([128, NQ * CCHUNK], f32, tag="xqt", name="xqt")
        nc.sync.dma_start(xqt[:], xq_d)
        wkt = P.tile([128, C * CCHUNK], fp16, tag="wkt", name="wkt")
        nc.sync.dma_start(wkt[:], wk_d)
        wqt = P.tile([128, C * CCHUNK], fp16, tag="wqt", name="wqt")
        nc.sync.dma_start(wqt[:], wq_d)
        wvt = P.tile([128, C * CCHUNK], fp16, tag="wvt", name="wvt")
        nc.sync.dma_start(wvt[:], wv_d)
        cntt = P.tile([128, 256 * NCHUNK], bf16, tag="cntt", name="cntt")
        nc.sync.dma_start(cntt[:], cnt_d)
        wpb = P.tile([64, C * HEADS], bf16, tag="wpb", name="wpb")
        nc.sync.dma_start(wpb[:], wp_d)

        h32 = [P.tile([128, N], fp16, tag=f"h32{k}", name=f"h32{k}")
               for k in range(CCHUNK)]
        hq32 = P.tile([128, NQ * CCHUNK], fp16, tag="hq32", name="hq32")

        def sm(k, f):
            return smallst[:, 5 * k + f:5 * k + f + 1]

        # ---- GroupNorm stats -> per-channel scale A / bias B ----
        s12 = [P.tile([128, 2], f32, tag=f"s12{k}", name=f"s12{k}")
               for k in range(CCHUNK)]
        At = [P.tile([128, 1], f32, tag=f"A{k}", name=f"A{k}")
              for k in range(CCHUNK)]
        Bt = [P.tile([128, 1], f32, tag=f"B{k}", name=f"B{k}")
              for k in range(CCHUNK)]
        with tc.tile_pool(name="gnps", bufs=1, space="PSUM") as gnps:
            # per-chunk sum (DVE reduce) / sum-of-squares (ACT Square
            # with free-axis accumulator), pipelined as chunks land
            for k in range(CCHUNK):
                nc.vector.tensor_reduce(s12[k][:, 0:1], xt[k][:],
                                        axis=AX.X, op=ALU.add)
                sq = A.tile([128, N], bf16, tag="sq", name="sq", bufs=2)
                nc.scalar.activation(sq[:], xt[k][:], AF.Square,
                                     accum_out=s12[k][:, 1:2])
            gs = gnps.tile([GROUPS, 2], f32, tag="gs", name="gs")
            for k in range(CCHUNK):
                for c2 in range(2):
                    nc.tensor.matmul(gs[:, c2:c2 + 1],
                                     gindt[:, 32 * k:32 * (k + 1)],
                                     s12[k][:, c2:c2 + 1],
                                     start=(k == 0 and c2 == 0),
                                     stop=(k == CCHUNK - 1 and c2 == 1),
                                     skip_group_check=True)
            mstat = P.tile([GROUPS, 2], f32, tag="mstat", name="mstat")
            inv_n = 1.0 / (GSIZE * N)
            nc.vector.tensor_scalar_mul(mstat[:, 0:1], gs[:, 0:1], inv_n)
            msq = P.tile([GROUPS, 1], f32, tag="msq", name="msq")
            nc.vector.tensor_scalar_mul(msq[:], gs[:, 1:2], inv_n)
            m2 = P.tile([GROUPS, 1], f32, tag="m2", name="m2")
            nc.vector.tensor_mul(m2[:], mstat[:, 0:1], mstat[:, 0:1])
            var = P.tile([GROUPS, 1], f32, tag="var", name="var")
            nc.vector.tensor_sub(var[:], msq[:], m2[:])
            nc.vector.tensor_scalar_add(var[:], var[:], float(EPS))
            std = P.tile([GROUPS, 1], f32, tag="std", name="std")
            nc.scalar.activation(std[:], var[:], AF.Sqrt)
            nc.vector.reciprocal(mstat[:, 1:2], std[:])
            mr = [P.tile([128, 2], f32, tag=f"mr{k}", name=f"mr{k}")
                  for k in range(CCHUNK)]
            for k in range(CCHUNK):
                mrp = gnps.tile([128, 2], f32, tag="mrp", name="mrp", bufs=2)
                nc.tensor.matmul(mrp[:], gindTt[:, 128 * k:128 * (k + 1)],
                                 mstat[:], start=True, stop=True)
                nc.vector.tensor_copy(mr[k][:], mrp[:])
            for k in range(CCHUNK):
                nc.vector.tensor_mul(At[k][:], sm(k, 3), mr[k][:, 1:2])
                tmp = P.tile([128, 1], f32, tag="tmpB", name="tmpB")
                nc.vector.tensor_mul(tmp[:], mr[k][:, 0:1], At[k][:])
                nc.vector.tensor_sub(Bt[k][:], sm(k, 4), tmp[:])
            # hq affine first (gates qconv), then h by column group so
            # kconv(0) can start after the first group (ACT/DVE split)
            for k in range(CCHUNK):
                nc.scalar.activation(hq32[:, NQ * k:NQ * (k + 1)],
                                     xqt[:, NQ * k:NQ * (k + 1)],
                                     AF.Identity,
                                     bias=Bt[k][:, 0:1], scale=At[k][:, 0:1])
            for j in range(4):
                for k in range(CCHUNK):
                    cols = slice(512 * j, 512 * (j + 1))
                    if (j + k) % 2 == 0:
                        nc.scalar.activation(h32[k][:, cols], xt[k][:, cols],
                                             AF.Identity, bias=Bt[k][:, 0:1],
                                             scale=At[k][:, 0:1])
                    else:
                        with nc.allow_low_precision(reason="f32r affine"):
                            nc.vector.tensor_scalar(
                                h32[k][:, cols], xt[k][:, cols],
                                At[k][:, 0:1], Bt[k][:, 0:1],
                                op0=ALU.mult, op1=ALU.add)

        # residual + proj bias, precombined for the tail epilogue
        xqb = P.tile([128, NQ * CCHUNK], f32, tag="xqb", name="xqb")
        for k in range(CCHUNK):
            nc.scalar.activation(xqb[:, NQ * k:NQ * (k + 1)],
                                 xqt[:, NQ * k:NQ * (k + 1)],
                                 AF.Identity, bias=sm(k, 2))

        kt = [P.tile([128, N], fp16, tag=f"kt{k}", name=f"kt{k}")
              for k in range(CCHUNK)]
        qt = [P.tile([128, NQ], fp16, tag=f"qt{k}", name=f"qt{k}")
              for k in range(CCHUNK)]
        vt = [P.tile([128, 65 * HEADS], bf16, tag=f"vt{m}", name=f"vt{m}")
              for m in range(NCHUNK)]
        on = P.tile([64, 256 * HEADS], bf16, tag="on", name="on")
        tln = P.tile([1, 256 * HEADS], f32, tag="tln", name="tln")
        rrb = P.tile([1, 256 * HEADS], bf16, tag="rrb", name="rrb")
        rbb = P.tile([64, 256 * HEADS], bf16, tag="rbb", name="rbb")
        ones1 = P.tile([1, 128], bf16, tag="ones1", name="ones1")
        nc.vector.memset(ones1[:], 1.0)
        LN2_32 = float(32 * np.log(2.0))
        bias_ln = P.tile([1, 1], f32, tag="bias_ln", name="bias_ln")
        nc.vector.memset(bias_ln[:], -LN2_32)

        def recip_chain(p, ot):
            # 1/rowsums: exp(-ln(s*2^-32) - 32ln2) on ACT. The 2^-32
            # scale keeps the Ln table input under its ~2^64 clip
            # (rowsums reach ~e^52).
            sl = slice(1024 * p, 1024 * (p + 1))
            nc.scalar.activation(tln[0:1, sl], ot[64:65, :], AF.Ln,
                                 scale=float(2.0 ** -32))
            nc.scalar.activation(rrb[0:1, sl], tln[0:1, sl], AF.Exp,
                                 scale=-1.0, bias=bias_ln[:, 0:1])

        # ---- fused conv + attention phase ----
        # One PSUM pool, 8 banks total: ot0 (2) + ot1 (2) + st (2) +
        # cp (2x1). Tail tiles reuse slots by tag (WAR-tracked).
        with tc.tile_pool(name="asb", bufs=3) as asb, \
                tc.tile_pool(name="aps", bufs=1, space="PSUM") as aps:
            ot0 = aps.tile([65, 256 * 4], f32, tag="ot0", name="ot0")
            ot1 = aps.tile([65, 256 * 4], f32, tag="ot1", name="ot1")

            def kchain(j, m):
                cols = slice(512 * j, 512 * (j + 1))
                pk = aps.tile([128, 512], f32, tag="cp", name="cpk", bufs=2)
                for ci in range(CCHUNK):
                    nc.tensor.matmul(
                        pk[:],
                        wkt[:, C * ci + 128 * m:C * ci + 128 * (m + 1)],
                        h32[ci][:, cols],
                        start=(ci == 0), stop=(ci == CCHUNK - 1))
                with nc.allow_low_precision(reason="f32r k"):
                    if m % 2 == 0:
                        nc.scalar.activation(kt[m][:, cols], pk[:],
                                             AF.Identity, bias=sm(m, 0))
                    else:
                        nc.vector.tensor_scalar_add(kt[m][:, cols],
                                                    pk[:], sm(m, 0))

            def qchain(m):
                pq = aps.tile([128, 512], f32, tag="cp", name="cpq",
                              bufs=2)[:, 0:NQ]
                for ci in range(CCHUNK):
                    nc.tensor.matmul(
                        pq[:],
                        wqt[:, C * ci + 128 * m:C * ci + 128 * (m + 1)],
                        hq32[:, NQ * ci:NQ * (ci + 1)],
                        start=(ci == 0), stop=(ci == CCHUNK - 1))
                with nc.allow_low_precision(reason="f32r q"):
                    nc.scalar.activation(qt[m][:], pq[:], AF.Identity,
                                         bias=sm(m, 1))

            def vconv(m):
                pv = aps.tile([128, C], f32, tag="cp", name="cpv", bufs=2)
                for ci in range(CCHUNK):
                    nc.tensor.matmul(pv[:],
                                     h32[ci][:, 128 * m:128 * (m + 1)],
                                     wvt[:, C * ci:C * (ci + 1)],
                                     start=(ci == 0),
                                     stop=(ci == CCHUNK - 1))
                dst = vt[m][:].rearrange("p (h e) -> p h e",
                                         h=HEADS)[:, :, 0:64]
                nc.vector.scalar_tensor_tensor(
                    dst, pv[:].rearrange("p (h d) -> p h d", h=HEADS), 1.0,
                    bvb[:].rearrange("p (h d) -> p h d", h=HEADS),
                    op0=ALU.mult, op1=ALU.add)
                ones_cols = vt[m][:].rearrange(
                    "p (h e) -> p h e", h=HEADS)[:, :, 64:65]
                nc.gpsimd.memset(ones_cols, 1.0)

            def attn_scores(p, m):
                st = aps.tile([128, 1024], f32, tag="st", name=f"st{p}_{m}")
                for h in range(4 * p, 4 * p + 4):
                    par = h % 2
                    cm = h // 2
                    lb = BLK[h] - 4 * p
                    nc.tensor.matmul(
                        st[:, 256 * lb:256 * (lb + 1)],
                        kt[cm][64 * par:64 * (par + 1),
                               128 * m:128 * (m + 1)],
                        qt[cm][64 * par:64 * (par + 1), :],
                        start=True, stop=True)
                et = asb.tile([128, 1024], bf16, tag="et", name=f"et{p}_{m}")
                nc.scalar.activation(et[:], st[:], AF.Exp)
                wt = asb.tile([128, 1024], bf16, tag="wt", name=f"wt{p}_{m}")
                nc.vector.tensor_mul(
                    wt[:].rearrange("p (b q) -> p b q", b=4),
                    et[:].rearrange("p (b q) -> p b q", b=4),
                    cntt[:, 256 * m:256 * (m + 1)].unsqueeze(1)
                        .broadcast_to([128, 4, NQ]))
                return wt

            ots = [ot0, ot1]

            def attn_ov(p, m, wt):
                for h in range(4 * p, 4 * p + 4):
                    lb = BLK[h] - 4 * p
                    nc.tensor.matmul(
                        ots[p][0:65, 256 * lb:256 * (lb + 1)],
                        vt[m][:, 65 * h:65 * h + 65],
                        wt[:, 256 * lb:256 * (lb + 1)],
                        start=(m == 0 and lb % 2 == 0),
                        stop=(m == NCHUNK - 1 and lb % 2 == 1),
                        skip_group_check=True)

            bvb = P.tile([128, C], f32, tag="bvb", name="bvb")
            prevs = [None, None]
            pm = [None, None]

            def attn_piece(p, m):
                wt = attn_scores(p, m)
                if prevs[p] is not None:
                    attn_ov(p, pm[p], prevs[p])
                prevs[p] = wt
                pm[p] = m

            # lead-in: group-0 convs + Q conv + bv broadcast
            for m in range(CCHUNK):
                kchain(0, m)
            for m in range(CCHUNK):
                qchain(m)
            pbv2 = aps.tile([128, C], f32, tag="cp", name="cpbv2", bufs=2)
            nc.tensor.matmul(pbv2[:], ones1[:], bvrow_t[:], start=True,
                             stop=True)
            nc.vector.tensor_copy(bvb[:], pbv2[:])
            for m in range(4):
                vconv(m)

            # weave: pass-0 chunks run a group behind their convs;
            # pass-1 chunks trail further so pass 0 finishes early and
            # its normalization chain overlaps pass 1's tail.
            zones = [
                ([(1, m) for m in range(CCHUNK)] + [(-1, m) for m in (4, 5, 6, 7)],
                 [(0, 0), (0, 1), (0, 2), (0, 3)]),
                ([(2, m) for m in range(CCHUNK)] + [(-1, m) for m in (8, 9, 10, 11)],
                 [(0, 4), (0, 5), (0, 6), (0, 7), (1, 0), (1, 1), (1, 2),
                  (1, 3)]),
                ([(3, m) for m in range(CCHUNK)] + [(-1, m) for m in (12, 13, 14, 15)],
                 [(0, 8), (0, 9), (0, 10), (0, 11), (1, 4), (1, 5), (1, 6),
                  (1, 7)]),
            ]
            for conv_list, attn_list in zones:
                na, nc_ = len(attn_list), len(conv_list)
                ai = 0
                for ci_, (cj, cm_) in enumerate(conv_list):
                    if cj >= 0:
                        kchain(cj, cm_)
                    else:
                        vconv(cm_)
                    while ai * nc_ < (ci_ + 1) * na:
                        p, m = attn_list[ai]
                        attn_piece(p, m)
                        ai += 1
                while ai < na:
                    p, m = attn_list[ai]
                    attn_piece(p, m)
                    ai += 1

            # zone 4: finish pass 0, start pass-0 normalization while
            # pass 1 keeps the ACT busy
            for p, m in [(0, 12), (0, 13), (0, 14), (0, 15)]:
                attn_piece(p, m)
            attn_ov(0, NCHUNK - 1, prevs[0])
            with nc.allow_low_precision(reason="bf16 recip"):
                nc.vector.reciprocal(rrb[0:1, 0:1024], ot0[64:65, :])
            for p, m in [(1, 8), (1, 9), (1, 10), (1, 11)]:
                attn_piece(p, m)
            # bcast0 into the two cp slots ([64,512] each)
            rb0 = [aps.tile([64, 512], f32, tag="cp", name=f"rb0_{jj}",
                            bufs=2) for jj in range(2)]
            for jj in range(2):
                nc.tensor.matmul(rb0[jj][:], ones1[0:1, 0:64],
                                 rrb[0:1, 512 * jj:512 * (jj + 1)],
                                 start=True, stop=True)
            with nc.allow_low_precision(reason="bf16 rb"):
                for jj in range(2):
                    nc.vector.tensor_copy(rbb[:, 512 * jj:512 * (jj + 1)],
                                          rb0[jj][:])
            with nc.allow_low_precision(reason="bf16 on"):
                nc.vector.tensor_mul(on[:, 0:1024], ot0[0:64, :],
                                     rbb[:, 0:1024])

            # zone 5: pass-1 tail chunks woven with the first half of
            # the projection (pjp pair tiles live in the cp slots)
            pjp = [aps.tile([128, 512], f32, tag="cp", name=f"pjp{pr}",
                            bufs=2) for pr in range(2)]
            pj = [pjp[mm // 2][:, 256 * (mm % 2):256 * (mm % 2 + 1)]
                  for mm in range(CCHUNK)]

            def proj_piece(mm, blo, bhi):
                for b in range(blo, bhi):
                    nc.tensor.matmul(
                        pj[mm],
                        wpb[:, C * b + 128 * mm:C * b + 128 * (mm + 1)],
                        on[:, 256 * b:256 * (b + 1)],
                        start=(b == 0 and mm % 2 == 0),
                        stop=(b == HEADS - 1 and mm % 2 == 1),
                        skip_group_check=True)

            z5 = [(1, 12), (1, 13), (1, 14), (1, 15)]
            for i, (p, m) in enumerate(z5):
                attn_piece(p, m)
                proj_piece(i, 0, 4)
            attn_ov(1, NCHUNK - 1, prevs[1])

            # tail: pass-1 normalization (ln/exp on ACT) + projection
            nc.scalar.activation(tln[0:1, 1024:2048], ot1[64:65, :],
                                 AF.Ln, scale=float(2.0 ** -32))
            nc.scalar.activation(rrb[0:1, 1024:2048], tln[0:1, 1024:2048],
                                 AF.Exp, scale=-1.0, bias=bias_ln[:, 0:1])
            rbp1 = aps.tile([64, 1024], f32, tag="ot0", name="rbp1")
            for jj in range(2):
                nc.tensor.matmul(rbp1[:, 512 * jj:512 * (jj + 1)],
                                 ones1[0:1, 0:64],
                                 rrb[0:1, 1024 + 512 * jj:
                                     1024 + 512 * (jj + 1)],
                                 start=True, stop=True)
            with nc.allow_low_precision(reason="bf16 rb"):
                nc.vector.tensor_copy(rbb[:, 1024:2048], rbp1[:])
            with nc.allow_low_precision(reason="bf16 on"):
                nc.vector.tensor_mul(on[:, 1024:2048], ot1[0:64, :],
                                     rbb[:, 1024:2048])
            with tc.tile_pool(name="psb", bufs=2) as psb:
                for pair in range(2):
                    for mm in (2 * pair, 2 * pair + 1):
                        proj_piece(mm, 4, HEADS)
                    outp = psb.tile([128, 2 * NQ], f32, tag="outp",
                                    name=f"outp{pair}")
                    nc.vector.tensor_add(outp[:], pjp[pair][:],
                                         xqb[:, 512 * pair:512 * (pair + 1)])
                    for mm in (2 * pair, 2 * pair + 1):
                        nc.sync.dma_start(
                            out_d[128 * mm:128 * (mm + 1), :],
                            outp[:, 256 * (mm - 2 * pair):
                                 256 * (mm - 2 * pair + 1)])

    if split_waits:
        _split_sync_waits(nc)
    return nc


# ---------------------------------------------------------------------------
# host-side input prep + entry point
# ---------------------------------------------------------------------------

def _tile_rows(a, p):
    """[K*p, F] -> [p, K*F] with row-chunk k at cols [k*F:(k+1)*F]."""
    kk = a.shape[0] // p
    return np.ascontiguousarray(
        a.reshape(kk, p, a.shape[1]).transpose(1, 0, 2).reshape(
            p, kk * a.shape[1]))


def _prep_inputs(x, valid_indices_mask, attendable_indices, gn_w, gn_b,
                 wq_, bq_, wk_, bk_, wv_, bv_, wp_, bp_):
    x = np.asarray(x, np.float32).reshape(C, N)
    idx = np.asarray(attendable_indices, np.int64)
    val = np.asarray(valid_indices_mask, np.float32)
    cnt_qn = np.zeros((N, N), np.float32)       # [q, n]
    rows = np.repeat(np.arange(N), K_IDX)
    np.add.at(cnt_qn, (rows, idx.reshape(-1)), val.reshape(-1))
    cntT = np.ascontiguousarray(cnt_qn.T).astype(ml_dtypes.bfloat16)  # [n, q]

    wq_ = np.asarray(wq_, np.float32)
    wk_ = np.asarray(wk_, np.float32)
    wv_ = np.asarray(wv_, np.float32)
    wp_ = np.asarray(wp_, np.float32)
    # wp column for o-channel (d*HEADS + h); our block order stacks head
    # HB[b] rows d-major at 64*b
    wpT = wp_.T                                    # [cin = d*8+h, cout]
    wpTb = np.empty((C, C), np.float32)
    for b in range(HEADS):
        h = HB[b]
        wpTb[64 * b:64 * (b + 1), :] = wpT[h::HEADS, :]

    gind = np.zeros((C, GROUPS), np.float32)
    gind[np.arange(C), np.arange(C) // GSIZE] = 1.0

    smalls = np.zeros((128, 20), np.float32)
    fields = [np.asarray(bk_, np.float32), np.asarray(bq_, np.float32),
              np.asarray(bp_, np.float32), np.asarray(gn_w, np.float32),
              np.asarray(gn_b, np.float32)]
    for k in range(CCHUNK):
        for f, arr in enumerate(fields):
            smalls[:, 5 * k + f] = arr.reshape(C)[128 * k:128 * (k + 1)]
    gind_all = np.zeros((128, 32 * CCHUNK), np.float32)
    for k in range(CCHUNK):
        gind_all[:, 32 * k:32 * (k + 1)] = gind[128 * k:128 * (k + 1), :]
    common = {
        "x": x.astype(ml_dtypes.bfloat16),
        "wk": _tile_rows(np.ascontiguousarray(wk_.T), 128).astype(np.float16),
        "wq": _tile_rows(np.ascontiguousarray(wq_.T), 128).astype(np.float16),
        "wv": _tile_rows(np.ascontiguousarray(wv_.T), 128).astype(np.float16),
        "wp": _tile_rows(wpTb, 64).astype(ml_dtypes.bfloat16),
        "smalls": smalls,
        "bvrow": np.asarray(bv_, np.float32).reshape(1, C)
            .astype(ml_dtypes.bfloat16),
        "gind": gind_all,
        "gindT": np.ascontiguousarray(gind.T),
    }
    in_maps = []
    for c in range(N_CORES):
        cols = slice(NQ * c, NQ * (c + 1))
        m = dict(common)
        m["xq"] = _tile_rows(np.ascontiguousarray(x[:, cols]), 128)
        m["cnt"] = _tile_rows(np.ascontiguousarray(cntT[:, cols]), 128)
        in_maps.append(m)
    return in_maps


def _enable_profile_hook():
    """Register the axon NTFF hook (this container's antenv lacks it)."""
    import antenv
    if 'antenv.axon_hooks' not in sys.modules:
        mod = types.ModuleType('antenv.axon_hooks')
        mod._hook = None
        mod.set_axon_ntff_profile_hook = lambda h: setattr(mod, '_hook', h)
        mod.get_axon_ntff_profile_hook = lambda: mod._hook
        sys.modules['antenv.axon_hooks'] = mod
        antenv.axon_hooks = mod
    from trn_agent_boot.trn_boot import _ntff_profile_via_ctypes
    sys.modules['antenv.axon_hooks'].set_axon_ntff_profile_hook(
        _ntff_profile_via_ctypes('/opt/axon/libaxon_pjrt.so'))
    import concourse.bass_utils as bu
    bu.upload_artifacts = lambda tmpdir: tmpdir


_CACHE = {}


def _run(inputs, trace=False):
    if "nc" not in _CACHE:
        _CACHE["nc"] = _build()
    nc = _CACHE["nc"]
    in_maps = _prep_inputs(
        inputs['x'], inputs['valid_indices_mask'],
        inputs['attendable_indices'], inputs['gn_w'], inputs['gn_b'],
        inputs['wq'], inputs['bq'], inputs['wk'], inputs['bk'],
        inputs['wv'], inputs['bv'], inputs['wp'], inputs['bp'])
    if trace:
        _enable_profile_hook()
    res = run_bass_kernel_spmd(nc, in_maps, list(range(N_CORES)), trace=trace)
    out = np.concatenate([res.results[c]["out"] for c in range(N_CORES)],
                         axis=1).reshape(1, C, N).astype(np.float32)
    return out, res


def kernel(**inputs):
    out, _ = _run(inputs, trace=False)
    return out
